# revision 1
# baseline (speedup 1.0000x reference)
"""Bass/Trainium2 kernel for nn_BayesianBertSelfAttention (B=2,S=1024,HID=768,NH=12,HD=64).

Sharding: 24 (batch, head) pairs over 8 cores -> core c handles batch c//4,
heads {3k, 3k+1, 3k+2} with k = c%4.

Per-core device algorithm (transposed-scores layout, scoresT[r, l]):
  phase P: q/k projections as 4 column-packed matmul groups (qT/kT [64, S]
           bf16), v projections directly in [r, d] layout (bf16, with a ones
           column producing softmax row sums via the context matmul).
  phase R (pipelined into S): relative-position table R'[l, c] = q . E_rev
           band per 128-row l-tile (bf16 matmul), copied bf16 to a DRAM
           scratch with row pitch 1152.
  phase S: per head: skewed bf16 read of R' gives bias[l, r] tiles (the
           Music-Transformer skew as a strided DRAM access pattern); PE
           transposes them to [r, l] in a bf16 psum; one DVE add fuses
           psum scores + bias -> bf16 SBUF. Dual softmax: ACT exp(scale=1/8)
           over a fused [128, 2048] global|local pair (local pre-multiplied
           by SM^T on GPSIMD). Unnormalized bf16 probs feed two context
           matmuls; the ones-column of v gives row sums in row 64.
  phase F: PE transposes ctxT back to [l, d], DVE normalizes (reciprocal of
           row sums), blends the two branches with selector weights, DMA out.

Host: packs weights/binds layouts, converts to bf16, reassembles [2,1024,768].
"""

import sys

sys.path.insert(0, "/opt/trn_rl_repo")

import numpy as np
import ml_dtypes
from contextlib import ExitStack

import concourse.bass as bass
import concourse.bacc as bacc
import concourse.tile as tile
from concourse import mybir
from concourse.bass_utils import run_bass_kernel_spmd
from concourse.masks import make_identity

B, S, HID, NH, HD = 2, 1024, 768, 12, 64
MAXP = 1024
NCORES = 8
HPC = 3            # heads per core
LTN = S // 128     # 8 l-tiles
BAND = 1151        # skew band width per 128-row l-tile
PITCH = 1152       # skew scratch row pitch
NE = 2 * MAXP - 1  # 2047

BF16 = mybir.dt.bfloat16
F32 = mybir.dt.float32
FP8 = mybir.dt.float8e4
COPY = mybir.ActivationFunctionType.Copy
EXP = mybir.ActivationFunctionType.Exp

NPBF16 = ml_dtypes.bfloat16

_programs = {}


def _bcast(ap, dim_count, insert_at):
    """Insert a step-0 broadcast dim of size dim_count at free-dim position."""
    new = list(ap.ap)
    new.insert(insert_at, [0, dim_count])
    return bass.AP(tensor=ap.tensor, offset=ap.offset, ap=new)


def build_program(n_cc=6, use_m=False):
    """n_cc: number of 128-row contraction chunks for projections (6 normally,
    7 when biases are nonzero and folded in as an extra ones row)."""
    nc = bacc.Bacc(None)
    CH = n_cc * 128

    hidT = nc.dram_tensor("hidT", [CH, S], BF16, kind="ExternalInput")
    wg = nc.dram_tensor("wg", [4, CH, 128], BF16, kind="ExternalInput")
    wv = nc.dram_tensor("wv", [CH, HPC * HD], BF16, kind="ExternalInput")
    embT2 = nc.dram_tensor("embT2", [128, NE], BF16, kind="ExternalInput")
    smT = nc.dram_tensor("smT", [S, S], BF16, kind="ExternalInput")
    selw = nc.dram_tensor("selw", [S, 2], F32, kind="ExternalInput")
    if use_m:
        mvec = nc.dram_tensor("mvec", [S, 2], F32, kind="ExternalInput")  # [m, 8m]
    outp = nc.dram_tensor("out", [S, HPC * HD], F32, kind="ExternalOutput")
    skews = [nc.dram_tensor(f"skew{h}", [LTN * 128 * PITCH], FP8)
             for h in range(HPC)]

    # (q_h, k_h) SBUF partition offsets per head; G-groups: 0=[q0|q1], 1=[k0|k1],
    # 2=[q2|-], 3=[k2|-]
    QG = [(0, 0), (0, 64), (2, 0)]   # (group, partition offset) for q
    KG = [(1, 0), (1, 64), (3, 0)]

    with tile.TileContext(nc) as tc, ExitStack() as ctx:
        singles = ctx.enter_context(tc.tile_pool(name="singles", bufs=1))

        hid_sb = singles.tile([128, n_cc, S], BF16)
        wg_sb = singles.tile([128, 4, n_cc, 128], BF16)
        wv_sb = singles.tile([128, n_cc, HPC * HD], BF16)
        emb_sb = singles.tile([128, NE], BF16)
        smT_sb = singles.tile([128, 8, S], BF16)
        selw_sb = singles.tile([128, 8, 2], F32)
        hid_v = hidT.rearrange("(cc p) l -> p cc l", p=128)
        wg_v = wg.rearrange("g (cc p) d -> p g cc d", p=128)
        nc.sync.dma_start(out=wg_sb[:, 0], in_=wg_v[:, 0])
        for cc in range(n_cc):
            nc.sync.dma_start(out=hid_sb[:, cc], in_=hid_v[:, cc])
        for g in range(1, 4):
            nc.sync.dma_start(out=wg_sb[:, g], in_=wg_v[:, g])
        nc.sync.dma_start(out=emb_sb, in_=embT2[:, :])
        nc.sync.dma_start(out=wv_sb, in_=wv.rearrange("(cc p) d -> p cc d", p=128))
        if use_m:
            m_sb = singles.tile([128, 8, 2], F32)
            nc.sync.dma_start(out=m_sb, in_=mvec.rearrange("(rs p) w -> p rs w", p=128))

        identB = singles.tile([128, 128], BF16)
        make_identity(nc, identB)
        identb = singles.tile([65, 65], BF16)
        make_identity(nc, identb)

        qkT_sb = singles.tile([128, 4, S], BF16)     # G-group projection outputs
        v4t_sb = singles.tile([128, 8, HPC * 65], BF16)  # v_aug per r-subtile
        cg_sb = singles.tile([65, HPC, S], BF16)     # unnormalized ctxT, global
        cl_sb = singles.tile([65, HPC, S], BF16)     # local

        # ---- phase P || R: projections + positional bands, one psum scope ----
        with tc.tile_pool(name="ps_pr", bufs=2, space="PSUM") as ps_pr, \
             tc.tile_pool(name="ps_tl", bufs=2, space="PSUM") as ps_tl, \
             tc.tile_pool(name="ps_pt", bufs=1, space="PSUM") as ps_pt, \
             tc.tile_pool(name="rsp", bufs=5) as rsp:

            def emit_G(g):
                mg = 128 if g < 2 else 64
                pt = ps_pt.tile([128, S], F32, tag="pt")
                for n in range(2):
                    for cc in range(n_cc):
                        nc.tensor.matmul(
                            pt[:mg, n * 512:(n + 1) * 512],
                            lhsT=wg_sb[:, g, cc, :mg],
                            rhs=hid_sb[:, cc, n * 512:(n + 1) * 512],
                            start=(cc == 0), stop=(cc == n_cc - 1),
                        )
                nc.scalar.activation(qkT_sb[:mg, g, :], pt[:mg], COPY)

            def emit_R(h):
                g, po = QG[h]
                qb = qkT_sb[po:po + 64, g, :]
                for lt in range(LTN):
                    pr = ps_pr.tile([128, S], F32, tag="big")
                    prt = ps_tl.tile([128, BAND - S], F32, tag="tail")
                    e0 = 896 - lt * 128
                    qbl = qb[:, lt * 128:(lt + 1) * 128]
                    for n0, n1 in ((0, 512), (512, 1024)):
                        nc.tensor.matmul(
                            pr[:, n0:n1],
                            lhsT=qbl,
                            rhs=emb_sb[po:po + 64, e0 + n0:e0 + n1],
                            start=True, stop=True,
                        )
                    nc.tensor.matmul(
                        prt, lhsT=qbl,
                        rhs=emb_sb[po:po + 64, e0 + S:e0 + BAND],
                        start=True, stop=True,
                    )
                    rt = rsp.tile([128, BAND], FP8, tag="rt")
                    if lt % 2 == 0:
                        nc.scalar.activation(rt[:, 0:S], pr, COPY)
                        nc.scalar.activation(rt[:, S:BAND], prt, COPY)
                    else:
                        nc.vector.tensor_copy(rt[:, 0:S], pr)
                        nc.vector.tensor_copy(rt[:, S:BAND], prt)
                    wview = skews[h][lt * 128 * PITCH:(lt + 1) * 128 * PITCH] \
                        .rearrange("(p c) -> p c", c=PITCH)[:, 0:BAND]
                    nc.sync.dma_start(out=wview, in_=rt)

            emit_G(0)
            emit_R(0)
            emit_G(1)
            emit_R(1)
            emit_G(2)
            emit_R(2)
            emit_G(3)
            nc.vector.memset(
                v4t_sb.rearrange("p rs (h x) -> p rs h x", x=65)[:, :, :, 64], 1.0
            )
            for rs in range(8):
                pv = ps_pt.tile([128, HPC * HD], F32, tag="pt")
                for cc in range(n_cc):
                    nc.tensor.matmul(
                        pv,
                        lhsT=hid_sb[:, cc, rs * 128:(rs + 1) * 128],
                        rhs=wv_sb[:, cc, :],
                        start=(cc == 0), stop=(cc == n_cc - 1),
                    )
                nc.vector.tensor_copy(
                    v4t_sb[:, rs, :].rearrange("p (h x) -> p h x", x=65)[:, :, 0:64],
                    pv.rearrange("p (h d) -> p h d", d=64),
                )

        smT_v = smT.rearrange("(rs p) l -> p rs l", p=128)
        for rs in range(8):
            nc.sync.dma_start(out=smT_sb[:, rs], in_=smT_v[:, rs])
        nc.sync.dma_start(out=selw_sb, in_=selw.rearrange("(lc p) w -> p lc w", p=128))

        # ---- phase S: scores + dual softmax + context (+ v projection) ----
        with tc.tile_pool(name="ps_s", bufs=1, space="PSUM") as ps_s, \
             tc.tile_pool(name="ps_bt", bufs=2, space="PSUM") as ps_bt, \
             tc.tile_pool(name="ps_cg", bufs=1, space="PSUM") as ps_cg, \
             tc.tile_pool(name="ps_cl", bufs=1, space="PSUM") as ps_cl, \
             tc.tile_pool(name="wk", bufs=6) as wk, \
             tc.tile_pool(name="bskp", bufs=2) as bskp:

            def emit_S(h):
                bsk8 = bskp.tile([128, LTN, S], FP8, tag="bsk8")
                bsk = bskp.tile([128, LTN, S], BF16, tag="bsk")
                for lt in range(LTN):
                    base = lt * 128 * PITCH
                    rview = skews[h][base + 127:base + 127 + 128 * BAND] \
                        .rearrange("(p c) -> p c", c=BAND)[:, 0:S]
                    nc.sync.dma_start(out=bsk8[:, lt, :], in_=rview)
                    nc.gpsimd.tensor_copy(bsk[:, lt, :], bsk8[:, lt, :])
                qg, qpo = QG[h]
                kg, kpo = KG[h]
                qf = qkT_sb[qpo:qpo + 64, qg, :]
                kf = qkT_sb[kpo:kpo + 64, kg, :]
                cg = ps_cg.tile([65, S], F32, tag="cg")
                cl = ps_cl.tile([65, S], F32, tag="cl")
                for rs in range(8):
                    # positional bias, transposed to [r, l] in a bf16 psum
                    bt = ps_bt.tile([128, S], BF16, tag="bt")
                    for lt in range(LTN):
                        nc.tensor.matmul(
                            bt[:, lt * 128:(lt + 1) * 128],
                            lhsT=bsk[:, lt, rs * 128:(rs + 1) * 128],
                            rhs=identB,
                            is_transpose=True, start=True, stop=True,
                        )
                    # raw scores (q.k), f32 psum
                    st = ps_s.tile([128, S], F32, tag="st")
                    for n in range(2):
                        nc.tensor.matmul(
                            st[:, n * 512:(n + 1) * 512],
                            lhsT=kf[:, rs * 128:(rs + 1) * 128],
                            rhs=qf[:, n * 512:(n + 1) * 512],
                            start=True, stop=True,
                        )
                    if use_m:
                        nc.vector.tensor_scalar_add(st, st, m_sb[:, rs, 1:2])
                    btc = wk.tile([128, S], BF16, tag="btc")
                    if rs % 3 == 2:
                        nc.scalar.activation(btc, bt, COPY)
                    else:
                        nc.vector.tensor_copy(btc, bt)
                    # sgtl = [scores+bias | (scores+bias)*smT], bf16
                    sgtl = wk.tile([128, 2 * S], BF16, tag="sgtl")
                    nc.vector.tensor_add(sgtl[:, 0:S], st, btc)
                    nc.gpsimd.tensor_mul(sgtl[:, S:2 * S], sgtl[:, 0:S],
                                         smT_sb[:, rs, :])
                    pgl = wk.tile([128, 2 * S], BF16, tag="pgl")
                    if use_m:
                        nc.scalar.activation(pgl[:, 0:S], sgtl[:, 0:S], EXP,
                                             scale=0.125)
                        nc.scalar.activation(pgl[:, S:2 * S], sgtl[:, S:2 * S],
                                             EXP, scale=0.125,
                                             bias=m_sb[:, rs, 0:1])
                    else:
                        nc.scalar.activation(pgl, sgtl, EXP, scale=0.125)
                    va = v4t_sb[:, rs, h * 65:(h + 1) * 65]
                    for n in range(2):
                        nc.tensor.matmul(
                            cg[:, n * 512:(n + 1) * 512],
                            lhsT=va, rhs=pgl[:, n * 512:(n + 1) * 512],
                            start=(rs == 0), stop=(rs == 7),
                        )
                        nc.tensor.matmul(
                            cl[:, n * 512:(n + 1) * 512],
                            lhsT=va, rhs=pgl[:, S + n * 512:S + (n + 1) * 512],
                            start=(rs == 0), stop=(rs == 7),
                        )
                nc.scalar.activation(cg_sb[:, h, :], cg, COPY)
                nc.vector.tensor_copy(cl_sb[:, h, :], cl)

            for h in range(HPC):
                emit_S(h)

        # ---- phase F: transpose back, normalize, blend, store ----
        with tc.tile_pool(name="ps_f", bufs=2, space="PSUM") as ps_f, \
             tc.tile_pool(name="fin", bufs=3) as fin:
            for lc in range(LTN):
                pf = ps_f.tile([128, 6 * 66], BF16, tag="pf")
                for h in range(HPC):
                    for br, csb in enumerate((cg_sb, cl_sb)):
                        x = h * 2 + br
                        nc.tensor.matmul(
                            pf[:, x * 66:x * 66 + 65],
                            lhsT=csb[:, h, lc * 128:(lc + 1) * 128],
                            rhs=identb,
                            is_transpose=True, start=True, stop=True,
                        )
                pfv = pf.rearrange("p (x c) -> p x c", c=66)
                rsum = fin.tile([128, 6], F32, tag="rsum")
                nc.vector.reciprocal(rsum, pfv[:, :, 64])
                w = fin.tile([128, 6], F32, tag="w")
                selv = selw_sb[:, lc, :]  # [128, 2]; col0=(1-sel) for g, col1=sel
                nc.vector.tensor_mul(
                    w.rearrange("p (h b) -> p h b", b=2),
                    rsum.rearrange("p (h b) -> p h b", b=2),
                    _bcast(selv, 3, 1),
                )
                tmp = fin.tile([128, 6, 64], F32, tag="tmp")
                nc.vector.tensor_mul(tmp, pfv[:, :, 0:64], _bcast(w, 64, 2))
                osb = fin.tile([128, HPC * HD], F32, tag="osb")
                tv = tmp.rearrange("p (h b) d -> p h b d", b=2)
                nc.vector.tensor_add(
                    osb.rearrange("p (h d) -> p h d", d=64),
                    tv[:, :, 0, :], tv[:, :, 1, :],
                )
                nc.sync.dma_start(out=outp[lc * 128:(lc + 1) * 128, :], in_=osb)

    nc.compile()
    return nc


def _get_program(n_cc, use_m):
    key = (n_cc, use_m)
    if key not in _programs:
        _programs[key] = build_program(n_cc, use_m)
    return _programs[key]


def kernel(hidden_states, attention_mask, scaled_attention_mask, selector_outputs,
           Wq, bq, Wk, bk, Wv, bv, dist_emb):
    hidden_states = np.asarray(hidden_states, np.float32)
    attention_mask = np.asarray(attention_mask, np.float32)
    scaled_attention_mask = np.asarray(scaled_attention_mask, np.float32)
    selector_outputs = np.asarray(selector_outputs, np.float32)
    Wq, Wk, Wv = (np.asarray(x, np.float32) for x in (Wq, Wk, Wv))
    bq, bk, bv = (np.asarray(x, np.float32) for x in (bq, bk, bv))
    dist_emb = np.asarray(dist_emb, np.float32)

    use_bias = bool(np.any(bq) or np.any(bk) or np.any(bv))
    use_m = bool(np.any(attention_mask))
    n_cc = 7 if use_bias else 6
    CH = n_cc * 128
    nc = _get_program(n_cc, use_m)

    smT = np.ascontiguousarray(scaled_attention_mask[0, 0].T).astype(NPBF16)
    e_rev_t = dist_emb[::-1].T.astype(NPBF16)
    embT2 = np.ascontiguousarray(np.concatenate([e_rev_t, e_rev_t], axis=0))

    in_maps = []
    for core in range(NCORES):
        b = core // 4
        k4 = core % 4
        heads = [3 * k4, 3 * k4 + 1, 3 * k4 + 2]

        hidT = hidden_states[b].T  # [768, S]
        if use_bias:
            hidT = np.concatenate(
                [hidT, np.ones((1, S), np.float32),
                 np.zeros((CH - HID - 1, S), np.float32)], axis=0)
        hidT_bf = np.ascontiguousarray(hidT).astype(NPBF16)

        def wcols(W, bvec, h):
            c = W[:, h * HD:(h + 1) * HD]
            if use_bias:
                c = np.concatenate(
                    [c, bvec[None, h * HD:(h + 1) * HD],
                     np.zeros((CH - HID - 1, HD), np.float32)], axis=0)
            return c

        q0, q1, q2 = (wcols(Wq, bq, h) for h in heads)
        k0, k1, k2 = (wcols(Wk, bk, h) for h in heads)
        z = np.zeros_like(q2)
        wg_np = np.stack([
            np.concatenate([q0, q1], axis=1),
            np.concatenate([k0, k1], axis=1),
            np.concatenate([q2, z], axis=1),
            np.concatenate([k2, z], axis=1),
        ]).astype(NPBF16)
        wv_np = np.concatenate(
            [wcols(Wv, bv, h) for h in heads], axis=1).astype(NPBF16)

        sel = selector_outputs[b, 0, :, 0]
        selw_np = np.stack([1.0 - sel, sel], axis=1).astype(np.float32)

        m = {
            "hidT": hidT_bf,
            "wg": wg_np,
            "wv": np.ascontiguousarray(wv_np),
            "embT2": embT2,
            "smT": smT,
            "selw": np.ascontiguousarray(selw_np),
        }
        if use_m:
            mv = attention_mask[b, 0, 0]
            m["mvec"] = np.ascontiguousarray(
                np.stack([mv, 8.0 * mv], axis=1).astype(np.float32))
        in_maps.append(m)

    res = run_bass_kernel_spmd(nc, in_maps, list(range(NCORES)))

    out = np.empty((B, S, HID), np.float32)
    for core in range(NCORES):
        b = core // 4
        k4 = core % 4
        out[b, :, 192 * k4:192 * (k4 + 1)] = res.results[core]["out"]
    return out



# revision 30
# speedup vs baseline: 1.3357x; 1.3357x over previous
"""Bass/Trainium2 kernel for nn_BayesianBertSelfAttention (B=2,S=1024,HID=768,NH=12,HD=64).

Sharding: 24 (batch, head) pairs over 8 cores -> core c handles batch c//4,
heads {3k, 3k+1, 3k+2} with k = c%4.

Per-core device algorithm (transposed-scores layout, scoresT[r, l]):
  phase P: q/k projections as 3 column-packed matmul groups ([q0|q1], [k0|k1],
           [q2|k2]); k2 re-based to partitions 0-63 via an SBUF->SBUF DMA.
           v projections in [r, d] layout (bf16).
  phase R (interleaved into S): relative-position band R'[l, c] = q . E_rev
           per 128-row l-tile (bf16 matmul), copied fp8 to a DRAM scratch with
           row pitch 1152; skewed fp8 read-back (Music-Transformer skew as a
           strided DRAM access pattern) gives bias[l, r] tiles in SBUF.
  phase S: per (head, r-chunk, l-half): the fp8 bias tiles are transposed AND
           added into the f32 score psum by regular fp8 matmuls with an
           identity rhs (out += bias_tile^T @ I), paired with a zeros slot so
           fp8 DoubleRow mode runs them at 2 rows/cycle. One psum->SBUF copy
           (ACT or DVE) materializes (s+b) bf16; GPSIMD multiplies by SM^T for
           the local branch; one fused ACT exp(scale=1/8) per r-chunk covers
           the [128, 2*1024] global|local pair. Context accumulated directly
           in [l, d] orientation: lhsT = probs chunk [r, l], rhs = v [r, d] ->
           psum [l, d] (plus a ones-column matmul for softmax row sums).
  phase F: per head, no transposes: DVE copies ctx psum to SBUF + reciprocal
           of row sums; GPSIMD blends the two branches with selector weights;
           per-head strided output DMA.

Host: packs weights/binds layouts, converts to bf16, reassembles [2,1024,768].
"""

import os
import sys

sys.path.insert(0, "/opt/trn_rl_repo")

import numpy as np
import ml_dtypes
from contextlib import ExitStack

import concourse.bass as bass
import concourse.bacc as bacc
import concourse.tile as tile
from concourse import mybir
from concourse.bass_utils import run_bass_kernel_spmd
from concourse.masks import make_identity

B, S, HID, NH, HD = 2, 1024, 768, 12, 64
MAXP = 1024
NCORES = 8
HPC = 3            # heads per core
LTN = S // 128     # 8 l-tiles
BAND = 1151        # skew band width per 128-row l-tile
PITCH = 1152       # skew scratch row pitch
NE = 2 * MAXP - 1  # 2047

BF16 = mybir.dt.bfloat16
F32 = mybir.dt.float32
FP8 = mybir.dt.float8e4
COPY = mybir.ActivationFunctionType.Copy
EXP = mybir.ActivationFunctionType.Exp
ADD = mybir.AluOpType.add

NPBF16 = ml_dtypes.bfloat16

_programs = {}

# engine-schedule knobs (sweepable via env for tuning)
RT_H0 = os.environ.get("K_RT_H0", "alt")     # head-0 rt copies: alt|act2
RT_HS = os.environ.get("K_RT_HS", "pi0")     # later heads: dve|pi0|pi2|pi02
STC_MOD = int(os.environ.get("K_STC_MOD", "0"))  # 1 in N stc halves on ACT (0=never)
R_SPREAD = os.environ.get("K_R_SPREAD", "front4")  # front4|all8


def _bcast(ap, dim_count, insert_at):
    """Insert a step-0 broadcast dim of size dim_count at free-dim position."""
    new = list(ap.ap)
    new.insert(insert_at, [0, dim_count])
    return bass.AP(tensor=ap.tensor, offset=ap.offset, ap=new)


def _dram_ap(ap, offset, dims):
    """Raw DRAM access pattern on ap's tensor (element units)."""
    return bass.AP(tensor=ap.tensor, offset=offset, ap=[list(d) for d in dims])


def build_program(n_cc=6, use_m=False):
    """n_cc: number of 128-row contraction chunks for projections (6 normally,
    7 when biases are nonzero and folded in as an extra ones row)."""
    nc = bacc.Bacc(None)
    CH = n_cc * 128

    hidT = nc.dram_tensor("hidT", [CH, S], BF16, kind="ExternalInput")
    wg = nc.dram_tensor("wg", [3, CH, 128], BF16, kind="ExternalInput")
    wv = nc.dram_tensor("wv", [CH, HPC * HD], BF16, kind="ExternalInput")
    embT2 = nc.dram_tensor("embT2", [128, NE], BF16, kind="ExternalInput")
    smT = nc.dram_tensor("smT", [S, S], BF16, kind="ExternalInput")
    selw = nc.dram_tensor("selw", [S, 2], F32, kind="ExternalInput")
    if use_m:
        mvec = nc.dram_tensor("mvec", [S, 2], F32, kind="ExternalInput")  # [m, 8m]
    outp = nc.dram_tensor("out", [S, HPC * HD], F32, kind="ExternalOutput")
    skews = [nc.dram_tensor(f"skew{h}", [LTN * 128 * PITCH], FP8)
             for h in range(HPC)]

    # (q_h, k_h) G-group/partition offsets: G0=[q0|q1], G1=[k0|k1], G2=[q2|k2]
    QG = [(0, 0), (0, 64), (2, 0)]
    KG = [(1, 0), (1, 64), None]  # head 2 k comes from k2lo_sb

    with tile.TileContext(nc) as tc, ExitStack() as ctx:
        singles = ctx.enter_context(tc.tile_pool(name="singles", bufs=1))

        hid_sb = singles.tile([128, n_cc, S], BF16)
        wg_sb = singles.tile([128, 3, n_cc, 128], BF16)
        wv_sb = singles.tile([128, n_cc, HPC * HD], BF16)
        emb_sb = singles.tile([128, NE], BF16)
        smT_sb = singles.tile([128, 8, S], BF16)
        selw_sb = singles.tile([128, 8, 2], F32)
        qkT_sb = singles.tile([128, 3, S], BF16)
        k2lo_sb = singles.tile([64, S], BF16)
        v4t_sb = singles.tile([128, 8, HPC * HD], BF16)
        ones1 = singles.tile([128, 1], BF16)
        id8z = singles.tile([128, 2, 128], FP8)  # [identity | zeros]
        osb = singles.tile([128, LTN, HPC * HD], F32)

        hid_v = hidT.rearrange("(cc p) l -> p cc l", p=128)
        wg_v = wg.rearrange("g (cc p) d -> p g cc d", p=128)
        # priority order on the serial DMA resource: weights for G0, hid,
        # emb (phase R), wg1/wv; the first smT quarter and wg2/smT-rest are
        # queued around head 0's skew round trip so S(0) can start early.
        nc.sync.dma_start(out=wg_sb[:, 0], in_=wg_v[:, 0])
        cch = n_cc // 2
        nc.sync.dma_start(out=hid_sb[:, 0:cch], in_=hid_v[:, 0:cch])
        nc.sync.dma_start(out=hid_sb[:, cch:n_cc], in_=hid_v[:, cch:n_cc])
        nc.sync.dma_start(out=wg_sb[:, 1], in_=wg_v[:, 1])
        nc.sync.dma_start(out=emb_sb, in_=embT2[:, :])
        nc.sync.dma_start(out=wv_sb, in_=wv.rearrange("(cc p) d -> p cc d", p=128))
        if use_m:
            m_sb = singles.tile([128, 8, 2], F32)
            nc.sync.dma_start(out=m_sb, in_=mvec.rearrange("(rs p) w -> p rs w", p=128))

        nc.vector.memset(ones1, 1.0)
        nc.vector.memset(id8z[:, 1, :], 0.0)
        make_identity(nc, id8z[:, 0, :])

        smT_v = smT.rearrange("(rs p) l -> p rs l", p=128)

        # R-phase psum pool lives across both P and S phases.
        ps_rb = ctx.enter_context(tc.tile_pool(name="ps_rb", bufs=3, space="PSUM"))
        rtp = ctx.enter_context(tc.tile_pool(name="rtp", bufs=3))
        bskp = ctx.enter_context(tc.tile_pool(name="bskp", bufs=3))
        sgtlp = ctx.enter_context(tc.tile_pool(name="sgtlp", bufs=3))
        pglp = ctx.enter_context(tc.tile_pool(name="pglp", bufs=3))
        finp = ctx.enter_context(tc.tile_pool(name="finp", bufs=2))

        rstate = {}
        ZS = LTN  # zeros slot index in bsk8 (for fp8 DoubleRow pairing)

        def r_begin(h):
            rt = rtp.tile([128, LTN, PITCH], FP8, tag="rt")
            bsk8 = bskp.tile([128, LTN + 1, S], FP8, tag="bsk")
            nc.gpsimd.memset(bsk8[:, ZS, :], 0.0)
            rstate[h] = (rt, bsk8)

        def r_unit(h, lt):
            """Band matmuls + psum->fp8 copies for one l-tile of head h.
            Head 0 runs in the P window where ACT is idle; later heads'
            copies go to DVE to keep ACT free for the exps."""
            rt, bsk8 = rstate[h]
            qg, qpo = QG[h]
            qb = qkT_sb[qpo:qpo + 64, qg, :]
            qbl = qb[:, lt * 128:(lt + 1) * 128]
            e0 = 896 - lt * 128
            pieces = ((0, 512), (512, 1024), (1024, BAND))
            for pi, (a, b) in enumerate(pieces):
                pr = ps_rb.tile([128, 512], F32, tag="rb")
                nc.tensor.matmul(
                    pr[:, 0:b - a], lhsT=qbl,
                    rhs=emb_sb[qpo:qpo + 64, e0 + a:e0 + b],
                    start=True, stop=True,
                )
                if h == 0:
                    if RT_H0 == "alt":
                        on_act = (lt + pi) % 2 == 0
                    else:
                        on_act = pi == 2 or (lt + pi) % 2 == 0
                elif RT_HS == "dve":
                    on_act = False
                elif RT_HS == "pi0":
                    on_act = pi == 0
                elif RT_HS == "pi2":
                    on_act = pi == 2
                elif RT_HS == "h1heavy":
                    on_act = pi in (0, 2) if h == 1 else pi == 0
                elif RT_HS == "h1all4":
                    if h == 1:
                        on_act = True if lt < 4 else pi in (0, 2)
                    else:
                        on_act = pi == 0
                else:
                    on_act = pi in (0, 2)
                if on_act:
                    nc.scalar.activation(rt[:, lt, a:b], pr[:, 0:b - a], COPY)
                else:
                    nc.vector.tensor_copy(rt[:, lt, a:b], pr[:, 0:b - a])
            if lt % 2 == 1:
                lt0 = lt - 1
                # skew write: rows (lt, l_loc), cols [0, BAND) with pitch PITCH
                wview = _dram_ap(
                    skews[h][:], lt0 * 128 * PITCH,
                    [[PITCH, 128], [128 * PITCH, 2], [1, BAND]])
                nc.sync.dma_start(out=wview, in_=rt[:, lt0:lt0 + 2, 0:BAND])
                # skewed read-back: bias[l, r] per l-tile
                rview = _dram_ap(
                    skews[h][:], lt0 * 128 * PITCH + 127,
                    [[BAND, 128], [128 * PITCH, 2], [1, S]])
                nc.sync.dma_start(out=bsk8[:, lt0:lt0 + 2, :], in_=rview)

        def emit_G(g, pool, tag="pt", on_act=True):
            for n in range(2):
                pt = pool.tile([128, 512], F32, tag=tag)
                for cc in range(n_cc):
                    nc.tensor.matmul(
                        pt,
                        lhsT=wg_sb[:, g, cc, :],
                        rhs=hid_sb[:, cc, n * 512:(n + 1) * 512],
                        start=(cc == 0), stop=(cc == n_cc - 1),
                    )
                if on_act:
                    nc.scalar.activation(qkT_sb[:, g, n * 512:(n + 1) * 512],
                                         pt, COPY)
                else:
                    nc.vector.tensor_copy(qkT_sb[:, g, n * 512:(n + 1) * 512],
                                          pt)

        # ---- phase P: projections (G2 deferred into S(0): wg2 loads late) ----
        with tc.tile_pool(name="ps_pt", bufs=2, space="PSUM") as ps_pt:
            emit_G(0, ps_pt)
            nc.sync.dma_start(out=smT_sb[:, 0:2], in_=smT_v[:, 0:2])
            for lt in range(LTN):
                if lt == 0:
                    r_begin(0)
                r_unit(0, lt)
            emit_G(1, ps_pt, on_act=False)
            nc.sync.dma_start(out=smT_sb[:, 2:8], in_=smT_v[:, 2:8])
            for rs in range(8):
                pv = ps_pt.tile([128, HPC * HD], F32, tag="pt")
                for cc in range(n_cc):
                    nc.tensor.matmul(
                        pv,
                        lhsT=hid_sb[:, cc, rs * 128:(rs + 1) * 128],
                        rhs=wv_sb[:, cc, :],
                        start=(cc == 0), stop=(cc == n_cc - 1),
                    )
                nc.vector.tensor_copy(v4t_sb[:, rs, :], pv)
            nc.sync.dma_start(out=wg_sb[:, 2], in_=wg_v[:, 2])
            nc.sync.dma_start(out=selw_sb,
                              in_=selw.rearrange("(lc p) w -> p lc w", p=128))

        # ---- phase S + F, with R(h+1) interleaved into S(h) ----
        with tc.tile_pool(name="ps_s", bufs=2, space="PSUM") as ps_s, \
             tc.tile_pool(name="ps_pc", bufs=1, space="PSUM") as ps_pc:

            id8z_pair = id8z[:, :, :]

            def bias_acc(st_slice, bsk8, lt, rs, stop):
                """st_slice += bias_tile^T via a regular fp8 matmul with an
                identity rhs; the zeros slot pairs the lhsT/rhs so fp8
                DoubleRow mode runs 2 rows/cycle."""
                base = bsk8[:, lt, rs * 128:(rs + 1) * 128]
                lhsT = bass.AP(
                    tensor=base.tensor, offset=base.offset,
                    ap=[list(base.ap[0]), [(ZS - lt) * S, 2]]
                       + [list(d) for d in base.ap[1:]])
                nc.tensor.matmul(
                    st_slice, lhsT=lhsT, rhs=id8z_pair,
                    start=False, stop=stop,
                    perf_mode=mybir.MatmulPerfMode.DoubleRow,
                )

            def s_front(h, rs):
                """scores + bias + (s+b) staging + local mul + fused exp ->
                pgl for one r-chunk."""
                _, bsk8 = rstate[h]
                qg, qpo = QG[h]
                qf = qkT_sb[qpo:qpo + 64, qg, :]
                if h < 2:
                    kg, kpo = KG[h]
                    kf = qkT_sb[kpo:kpo + 64, kg, :]
                else:
                    kf = k2lo_sb[:, :]
                sgtl = sgtlp.tile([128, 2, S], BF16, tag="sgtl")
                for half in range(2):
                    st = ps_s.tile([128, 512], F32, tag="st")
                    nc.tensor.matmul(
                        st,
                        lhsT=kf[:, rs * 128:(rs + 1) * 128],
                        rhs=qf[:, half * 512:(half + 1) * 512],
                        start=True, stop=False,
                    )
                    for lc4 in range(4):
                        bias_acc(st[:, lc4 * 128:(lc4 + 1) * 128],
                                 bsk8, half * 4 + lc4, rs,
                                 stop=(lc4 == 3))
                    sg = sgtl[:, 0, half * 512:(half + 1) * 512]
                    # (s+b) psum -> SBUF bf16; mostly DVE (ACT is exp-bound)
                    stc_act = STC_MOD > 0 and \
                        (h * 16 + rs * 2 + half) % STC_MOD == 0
                    if use_m:
                        if stc_act:
                            nc.scalar.activation(
                                sg, st,
                                mybir.ActivationFunctionType.Identity,
                                bias=m_sb[:, rs, 1:2])
                        else:
                            nc.vector.tensor_scalar_add(
                                sg, st, m_sb[:, rs, 1:2])
                    elif stc_act:
                        nc.scalar.activation(sg, st, COPY)
                    else:
                        nc.vector.tensor_copy(sg, st)
                    nc.gpsimd.tensor_mul(
                        sgtl[:, 1, half * 512:(half + 1) * 512], sg,
                        smT_sb[:, rs, half * 512:(half + 1) * 512])
                pgl = pglp.tile([128, 2, S], BF16, tag="pgl")
                if use_m:
                    nc.scalar.activation(pgl[:, 0, :], sgtl[:, 0, :],
                                         EXP, scale=0.125)
                    nc.scalar.activation(pgl[:, 1, :], sgtl[:, 1, :],
                                         EXP, scale=0.125,
                                         bias=m_sb[:, rs, 0:1])
                else:
                    nc.scalar.activation(pgl, sgtl, EXP, scale=0.125)
                return pgl

            def emit_S(h, front0=None):
                vv = v4t_sb.rearrange("p rs (h d) -> p rs h d", d=HD)[:, :, h, :]
                pcv = ps_pc.tile([128, LTN, 2, HD], F32, tag="pcv")
                pcs = ps_pc.tile([128, LTN, 2, 1], F32, tag="pcs")
                next_front = None
                for rs in range(8):
                    pgl = front0 if (rs == 0 and front0 is not None) \
                        else s_front(h, rs)
                    if rs == 7 and h + 1 < HPC:
                        # cross-head pipelining: next head's first front runs
                        # its exp while this head's last ctx waits
                        next_front = s_front(h + 1, 0)
                    # pcv spans 2 psum banks (lt 0-3 / 4-7); pcs one more.
                    # start_tensor_calc arms zero-on-write per 2KB bank, so
                    # exactly one start per bank and one stop at the bank's
                    # last matmul (PE executes in emission order).
                    for br in range(2):
                        for lt in range(LTN):
                            lw = pgl[:, br, lt * 128:(lt + 1) * 128]
                            nc.tensor.matmul(
                                pcv[:, lt, br, :], lhsT=lw,
                                rhs=vv[:, rs, :],
                                start=(rs == 0 and br == 0 and lt % 4 == 0),
                                stop=(rs == 7 and br == 1 and lt % 4 == 3),
                                skip_group_check=True,
                            )
                            nc.tensor.matmul(
                                pcs[:, lt, br, :], lhsT=lw,
                                rhs=ones1,
                                start=(rs == 0 and br == 0 and lt == 0),
                                stop=(rs == 7 and br == 1 and lt == 7),
                                skip_group_check=True,
                            )
                    if h == 0 and rs == 1:
                        emit_G(2, ps_s, tag="st", on_act=False)
                        # k2 re-based to partitions 0-63 (engines cannot
                        # cross partitions)
                        nc.sync.dma_start(out=k2lo_sb, in_=qkT_sb[64:128, 2, :])
                    if h + 1 < HPC and rs < 4:
                        if rs == 0:
                            r_begin(h + 1)
                        r_unit(h + 1, 2 * rs)
                        r_unit(h + 1, 2 * rs + 1)
                return pcv, pcs, next_front

            def emit_F(h, pcv, pcs):
                pcb = finp.tile([128, LTN, 2, HD], F32, tag="pcb")
                rsum = finp.tile([128, LTN, 2], F32, tag="rsum")
                w = finp.tile([128, LTN, 2], F32, tag="w")
                tmp = finp.tile([128, LTN, HD], F32, tag="tmp")
                tmp2 = finp.tile([128, LTN, HD], F32, tag="tmp2")
                ov = osb.rearrange("p lc (h d) -> p lc h d", d=HD)[:, :, h, :]
                for q in range(2):
                    a, b = q * 4, q * 4 + 4
                    nc.vector.tensor_copy(pcb[:, a:b], pcv[:, a:b])
                    nc.vector.reciprocal(rsum[:, a:b], pcs[:, a:b, :, 0])
                    nc.gpsimd.tensor_mul(w[:, a:b], rsum[:, a:b],
                                         selw_sb[:, a:b])
                    nc.gpsimd.tensor_mul(tmp[:, a:b], pcb[:, a:b, 1, :],
                                         _bcast(w[:, a:b, 1], HD, 2))
                    nc.gpsimd.tensor_mul(tmp2[:, a:b], pcb[:, a:b, 0, :],
                                         _bcast(w[:, a:b, 0], HD, 2))
                    nc.gpsimd.tensor_add(ov[:, a:b], tmp[:, a:b],
                                         tmp2[:, a:b])
                    outv = outp.rearrange("(lc p) c -> p lc c", p=128)
                    nc.sync.dma_start(
                        out=outv[:, a:b, h * HD:(h + 1) * HD],
                        in_=osb[:, a:b, h * HD:(h + 1) * HD])

            front = None
            for h in range(HPC):
                pcv, pcs, front = emit_S(h, front)
                emit_F(h, pcv, pcs)

    nc.compile()
    return nc


def _get_program(n_cc, use_m):
    key = (n_cc, use_m)
    if key not in _programs:
        _programs[key] = build_program(n_cc, use_m)
    return _programs[key]


def kernel(hidden_states, attention_mask, scaled_attention_mask, selector_outputs,
           Wq, bq, Wk, bk, Wv, bv, dist_emb):
    hidden_states = np.asarray(hidden_states, np.float32)
    attention_mask = np.asarray(attention_mask, np.float32)
    scaled_attention_mask = np.asarray(scaled_attention_mask, np.float32)
    selector_outputs = np.asarray(selector_outputs, np.float32)
    Wq, Wk, Wv = (np.asarray(x, np.float32) for x in (Wq, Wk, Wv))
    bq, bk, bv = (np.asarray(x, np.float32) for x in (bq, bk, bv))
    dist_emb = np.asarray(dist_emb, np.float32)

    use_bias = bool(np.any(bq) or np.any(bk) or np.any(bv))
    use_m = bool(np.any(attention_mask))
    n_cc = 7 if use_bias else 6
    CH = n_cc * 128
    nc = _get_program(n_cc, use_m)

    smT = np.ascontiguousarray(scaled_attention_mask[0, 0].T).astype(NPBF16)
    e_rev_t = dist_emb[::-1].T.astype(NPBF16)
    embT2 = np.ascontiguousarray(np.concatenate([e_rev_t, e_rev_t], axis=0))

    in_maps = []
    for core in range(NCORES):
        b = core // 4
        k4 = core % 4
        heads = [3 * k4, 3 * k4 + 1, 3 * k4 + 2]

        hidT = hidden_states[b].T  # [768, S]
        if use_bias:
            hidT = np.concatenate(
                [hidT, np.ones((1, S), np.float32),
                 np.zeros((CH - HID - 1, S), np.float32)], axis=0)
        hidT_bf = np.ascontiguousarray(hidT).astype(NPBF16)

        def wcols(W, bvec, h):
            c = W[:, h * HD:(h + 1) * HD]
            if use_bias:
                c = np.concatenate(
                    [c, bvec[None, h * HD:(h + 1) * HD],
                     np.zeros((CH - HID - 1, HD), np.float32)], axis=0)
            return c

        q0, q1, q2 = (wcols(Wq, bq, h) for h in heads)
        k0, k1, k2 = (wcols(Wk, bk, h) for h in heads)
        wg_np = np.stack([
            np.concatenate([q0, q1], axis=1),
            np.concatenate([k0, k1], axis=1),
            np.concatenate([q2, k2], axis=1),
        ]).astype(NPBF16)
        wv_np = np.concatenate(
            [wcols(Wv, bv, h) for h in heads], axis=1).astype(NPBF16)

        sel = selector_outputs[b, 0, :, 0]
        selw_np = np.stack([1.0 - sel, sel], axis=1).astype(np.float32)

        m = {
            "hidT": hidT_bf,
            "wg": wg_np,
            "wv": np.ascontiguousarray(wv_np),
            "embT2": embT2,
            "smT": smT,
            "selw": np.ascontiguousarray(selw_np),
        }
        if use_m:
            mv = attention_mask[b, 0, 0]
            m["mvec"] = np.ascontiguousarray(
                np.stack([mv, 8.0 * mv], axis=1).astype(np.float32))
        in_maps.append(m)

    res = run_bass_kernel_spmd(nc, in_maps, list(range(NCORES)))

    out = np.empty((B, S, HID), np.float32)
    for core in range(NCORES):
        b = core // 4
        k4 = core % 4
        out[b, :, 192 * k4:192 * (k4 + 1)] = res.results[core]["out"]
    return out


# revision 32
# speedup vs baseline: 1.3624x; 1.0200x over previous
"""Bass/Trainium2 kernel for nn_BayesianBertSelfAttention (B=2,S=1024,HID=768,NH=12,HD=64).

Sharding: 24 (batch, head) pairs over 8 cores -> core c handles batch c//4,
heads {3k, 3k+1, 3k+2} with k = c%4.

Per-core device algorithm (transposed-scores layout, scoresT[r, l]):
  phase P: q/k projections as 3 column-packed matmul groups ([q0|q1], [k0|k1],
           [q2|k2]); k2 re-based to partitions 0-63 via an SBUF->SBUF DMA.
           v projections in [r, d] layout (bf16).
  phase R (interleaved into S): relative-position band R'[l, c] = q . E_rev
           per 128-row l-tile (bf16 matmul), copied fp8 to a DRAM scratch with
           row pitch 1152; skewed fp8 read-back (Music-Transformer skew as a
           strided DRAM access pattern) gives bias[l, r] tiles in SBUF.
  phase S: per (head, r-chunk, l-half): the fp8 bias tiles are transposed AND
           added into the f32 score psum by regular fp8 matmuls with an
           identity rhs (out += bias_tile^T @ I), paired with a zeros slot so
           fp8 DoubleRow mode runs them at 2 rows/cycle. One psum->SBUF copy
           (ACT or DVE) materializes (s+b) bf16; GPSIMD multiplies by SM^T for
           the local branch; one fused ACT exp(scale=1/8) per r-chunk covers
           the [128, 2*1024] global|local pair. Context accumulated directly
           in [l, d] orientation: lhsT = probs chunk [r, l], rhs = v [r, d] ->
           psum [l, d] (plus a ones-column matmul for softmax row sums).
  phase F: per head, no transposes: DVE copies ctx psum to SBUF + reciprocal
           of row sums; GPSIMD blends the two branches with selector weights;
           per-head strided output DMA.

Host: packs weights/binds layouts, converts to bf16, reassembles [2,1024,768].
"""

import os
import sys

sys.path.insert(0, "/opt/trn_rl_repo")

import numpy as np
import ml_dtypes
from contextlib import ExitStack

import concourse.bass as bass
import concourse.bacc as bacc
import concourse.tile as tile
from concourse import mybir
from concourse.bass_utils import run_bass_kernel_spmd
from concourse.masks import make_identity

B, S, HID, NH, HD = 2, 1024, 768, 12, 64
MAXP = 1024
NCORES = 8
HPC = 3            # heads per core
LTN = S // 128     # 8 l-tiles
BAND = 1151        # skew band width per 128-row l-tile
PITCH = 1152       # skew scratch row pitch
NE = 2 * MAXP - 1  # 2047

BF16 = mybir.dt.bfloat16
F32 = mybir.dt.float32
FP8 = mybir.dt.float8e4
COPY = mybir.ActivationFunctionType.Copy
EXP = mybir.ActivationFunctionType.Exp
ADD = mybir.AluOpType.add

NPBF16 = ml_dtypes.bfloat16

_programs = {}

# engine-schedule knobs (sweepable via env for tuning)
RT_H0 = os.environ.get("K_RT_H0", "alt")     # head-0 rt copies: alt|act2
RT_HS = os.environ.get("K_RT_HS", "h1heavy")  # later heads: dve|pi0|pi2|pi02|h1heavy|h1all4
STC_MOD = int(os.environ.get("K_STC_MOD", "0"))  # 1 in N stc halves on ACT (0=never)
R_SPREAD = os.environ.get("K_R_SPREAD", "front4")  # front4|all8


def _bcast(ap, dim_count, insert_at):
    """Insert a step-0 broadcast dim of size dim_count at free-dim position."""
    new = list(ap.ap)
    new.insert(insert_at, [0, dim_count])
    return bass.AP(tensor=ap.tensor, offset=ap.offset, ap=new)


def _dram_ap(ap, offset, dims):
    """Raw DRAM access pattern on ap's tensor (element units)."""
    return bass.AP(tensor=ap.tensor, offset=offset, ap=[list(d) for d in dims])


def build_program(n_cc=6, use_m=False):
    """n_cc: number of 128-row contraction chunks for projections (6 normally,
    7 when biases are nonzero and folded in as an extra ones row)."""
    nc = bacc.Bacc(None)
    CH = n_cc * 128

    hidT = nc.dram_tensor("hidT", [CH, S], BF16, kind="ExternalInput")
    wg = nc.dram_tensor("wg", [3, CH, 128], BF16, kind="ExternalInput")
    wv = nc.dram_tensor("wv", [CH, HPC * HD], BF16, kind="ExternalInput")
    embT2 = nc.dram_tensor("embT2", [128, NE], BF16, kind="ExternalInput")
    smT = nc.dram_tensor("smT", [S, S], FP8, kind="ExternalInput")
    selw = nc.dram_tensor("selw", [S, 2], F32, kind="ExternalInput")
    if use_m:
        mvec = nc.dram_tensor("mvec", [S, 2], F32, kind="ExternalInput")  # [m, 8m]
    outp = nc.dram_tensor("out", [S, HPC * HD], F32, kind="ExternalOutput")
    skews = [nc.dram_tensor(f"skew{h}", [LTN * 128 * PITCH], FP8)
             for h in range(HPC)]

    # (q_h, k_h) G-group/partition offsets: G0=[q0|q1], G1=[k0|k1], G2=[q2|k2]
    QG = [(0, 0), (0, 64), (2, 0)]
    KG = [(1, 0), (1, 64), None]  # head 2 k comes from k2lo_sb

    with tile.TileContext(nc) as tc, ExitStack() as ctx:
        singles = ctx.enter_context(tc.tile_pool(name="singles", bufs=1))

        hid_sb = singles.tile([128, n_cc, S], BF16)
        wg_sb = singles.tile([128, 3, n_cc, 128], BF16)
        wv_sb = singles.tile([128, n_cc, HPC * HD], BF16)
        emb_sb = singles.tile([128, NE], BF16)
        smT_sb = singles.tile([128, 8, S], FP8)
        selw_sb = singles.tile([128, 8, 2], F32)
        qkT_sb = singles.tile([128, 3, S], BF16)
        k2lo_sb = singles.tile([64, S], BF16)
        v4t_sb = singles.tile([128, 8, HPC * HD], BF16)
        ones1 = singles.tile([128, 1], BF16)
        id8z = singles.tile([128, 2, 128], FP8)  # [identity | zeros]
        osb = singles.tile([128, LTN, HPC * HD], F32)

        hid_v = hidT.rearrange("(cc p) l -> p cc l", p=128)
        wg_v = wg.rearrange("g (cc p) d -> p g cc d", p=128)
        # priority order on the serial DMA resource: weights for G0, hid,
        # emb (phase R), wg1/wv; the first smT quarter and wg2/smT-rest are
        # queued around head 0's skew round trip so S(0) can start early.
        nc.sync.dma_start(out=wg_sb[:, 0], in_=wg_v[:, 0])
        cch = n_cc // 2
        nc.sync.dma_start(out=hid_sb[:, 0:cch], in_=hid_v[:, 0:cch])
        nc.sync.dma_start(out=hid_sb[:, cch:n_cc], in_=hid_v[:, cch:n_cc])
        nc.sync.dma_start(out=wg_sb[:, 1], in_=wg_v[:, 1])
        nc.sync.dma_start(out=emb_sb, in_=embT2[:, :])
        nc.sync.dma_start(out=wv_sb, in_=wv.rearrange("(cc p) d -> p cc d", p=128))
        if use_m:
            m_sb = singles.tile([128, 8, 2], F32)
            nc.sync.dma_start(out=m_sb, in_=mvec.rearrange("(rs p) w -> p rs w", p=128))

        nc.vector.memset(ones1, 1.0)
        nc.vector.memset(id8z[:, 1, :], 0.0)
        make_identity(nc, id8z[:, 0, :])

        smT_v = smT.rearrange("(rs p) l -> p rs l", p=128)

        # R-phase psum pool lives across both P and S phases.
        ps_rb = ctx.enter_context(tc.tile_pool(name="ps_rb", bufs=3, space="PSUM"))
        rtp = ctx.enter_context(tc.tile_pool(name="rtp", bufs=3))
        bskp = ctx.enter_context(tc.tile_pool(name="bskp", bufs=3))
        sgtlp = ctx.enter_context(tc.tile_pool(name="sgtlp", bufs=3))
        pglp = ctx.enter_context(tc.tile_pool(name="pglp", bufs=3))
        finp = ctx.enter_context(tc.tile_pool(name="finp", bufs=2))

        rstate = {}
        ZS = LTN  # zeros slot index in bsk8 (for fp8 DoubleRow pairing)

        def r_begin(h):
            rt = rtp.tile([128, LTN, PITCH], FP8, tag="rt")
            bsk8 = bskp.tile([128, LTN + 1, S], FP8, tag="bsk")
            nc.gpsimd.memset(bsk8[:, ZS, :], 0.0)
            rstate[h] = (rt, bsk8)

        def r_unit(h, lt):
            """Band matmuls + psum->fp8 copies for one l-tile of head h.
            Head 0 runs in the P window where ACT is idle; later heads'
            copies go to DVE to keep ACT free for the exps."""
            rt, bsk8 = rstate[h]
            qg, qpo = QG[h]
            qb = qkT_sb[qpo:qpo + 64, qg, :]
            qbl = qb[:, lt * 128:(lt + 1) * 128]
            e0 = 896 - lt * 128
            pieces = ((0, 512), (512, 1024), (1024, BAND))
            for pi, (a, b) in enumerate(pieces):
                pr = ps_rb.tile([128, 512], F32, tag="rb")
                nc.tensor.matmul(
                    pr[:, 0:b - a], lhsT=qbl,
                    rhs=emb_sb[qpo:qpo + 64, e0 + a:e0 + b],
                    start=True, stop=True,
                )
                if h == 0:
                    if RT_H0 == "alt":
                        on_act = (lt + pi) % 2 == 0
                    else:
                        on_act = pi == 2 or (lt + pi) % 2 == 0
                elif RT_HS == "dve":
                    on_act = False
                elif RT_HS == "pi0":
                    on_act = pi == 0
                elif RT_HS == "pi2":
                    on_act = pi == 2
                elif RT_HS == "h1heavy":
                    on_act = pi in (0, 2) if h == 1 else pi == 0
                elif RT_HS == "h1all4":
                    if h == 1:
                        on_act = True if lt < 4 else pi in (0, 2)
                    else:
                        on_act = pi == 0
                else:
                    on_act = pi in (0, 2)
                if on_act:
                    nc.scalar.activation(rt[:, lt, a:b], pr[:, 0:b - a], COPY)
                else:
                    nc.vector.tensor_copy(rt[:, lt, a:b], pr[:, 0:b - a])
            if lt % 2 == 1:
                lt0 = lt - 1
                # skew write: rows (lt, l_loc), cols [0, BAND) with pitch PITCH
                wview = _dram_ap(
                    skews[h][:], lt0 * 128 * PITCH,
                    [[PITCH, 128], [128 * PITCH, 2], [1, BAND]])
                nc.sync.dma_start(out=wview, in_=rt[:, lt0:lt0 + 2, 0:BAND])
                # skewed read-back: bias[l, r] per l-tile
                rview = _dram_ap(
                    skews[h][:], lt0 * 128 * PITCH + 127,
                    [[BAND, 128], [128 * PITCH, 2], [1, S]])
                nc.sync.dma_start(out=bsk8[:, lt0:lt0 + 2, :], in_=rview)

        def emit_G(g, pool, tag="pt", on_act=True):
            for n in range(2):
                pt = pool.tile([128, 512], F32, tag=tag)
                for cc in range(n_cc):
                    nc.tensor.matmul(
                        pt,
                        lhsT=wg_sb[:, g, cc, :],
                        rhs=hid_sb[:, cc, n * 512:(n + 1) * 512],
                        start=(cc == 0), stop=(cc == n_cc - 1),
                    )
                if on_act:
                    nc.scalar.activation(qkT_sb[:, g, n * 512:(n + 1) * 512],
                                         pt, COPY)
                else:
                    nc.vector.tensor_copy(qkT_sb[:, g, n * 512:(n + 1) * 512],
                                          pt)

        # ---- phase P: projections (G2 deferred into S(0): wg2 loads late) ----
        with tc.tile_pool(name="ps_pt", bufs=2, space="PSUM") as ps_pt:
            emit_G(0, ps_pt)
            nc.sync.dma_start(out=smT_sb[:, 0:2], in_=smT_v[:, 0:2])
            for lt in range(LTN):
                if lt == 0:
                    r_begin(0)
                r_unit(0, lt)
            emit_G(1, ps_pt, on_act=False)
            nc.sync.dma_start(out=smT_sb[:, 2:8], in_=smT_v[:, 2:8])
            for rs in range(8):
                pv = ps_pt.tile([128, HPC * HD], F32, tag="pt")
                for cc in range(n_cc):
                    nc.tensor.matmul(
                        pv,
                        lhsT=hid_sb[:, cc, rs * 128:(rs + 1) * 128],
                        rhs=wv_sb[:, cc, :],
                        start=(cc == 0), stop=(cc == n_cc - 1),
                    )
                nc.vector.tensor_copy(v4t_sb[:, rs, :], pv)
            nc.sync.dma_start(out=wg_sb[:, 2], in_=wg_v[:, 2])
            nc.sync.dma_start(out=selw_sb,
                              in_=selw.rearrange("(lc p) w -> p lc w", p=128))

        # ---- phase S + F, with R(h+1) interleaved into S(h) ----
        with tc.tile_pool(name="ps_s", bufs=2, space="PSUM") as ps_s, \
             tc.tile_pool(name="ps_pc", bufs=1, space="PSUM") as ps_pc:

            id8z_pair = id8z[:, :, :]

            def bias_acc(st_slice, bsk8, lt, rs, stop):
                """st_slice += bias_tile^T via a regular fp8 matmul with an
                identity rhs; the zeros slot pairs the lhsT/rhs so fp8
                DoubleRow mode runs 2 rows/cycle."""
                base = bsk8[:, lt, rs * 128:(rs + 1) * 128]
                lhsT = bass.AP(
                    tensor=base.tensor, offset=base.offset,
                    ap=[list(base.ap[0]), [(ZS - lt) * S, 2]]
                       + [list(d) for d in base.ap[1:]])
                nc.tensor.matmul(
                    st_slice, lhsT=lhsT, rhs=id8z_pair,
                    start=False, stop=stop,
                    perf_mode=mybir.MatmulPerfMode.DoubleRow,
                )

            def s_front(h, rs):
                """scores + bias + (s+b) staging + local mul + fused exp ->
                pgl for one r-chunk."""
                _, bsk8 = rstate[h]
                qg, qpo = QG[h]
                qf = qkT_sb[qpo:qpo + 64, qg, :]
                if h < 2:
                    kg, kpo = KG[h]
                    kf = qkT_sb[kpo:kpo + 64, kg, :]
                else:
                    kf = k2lo_sb[:, :]
                sgtl = sgtlp.tile([128, 2, S], BF16, tag="sgtl")
                for half in range(2):
                    st = ps_s.tile([128, 512], F32, tag="st")
                    nc.tensor.matmul(
                        st,
                        lhsT=kf[:, rs * 128:(rs + 1) * 128],
                        rhs=qf[:, half * 512:(half + 1) * 512],
                        start=True, stop=False,
                    )
                    for lc4 in range(4):
                        bias_acc(st[:, lc4 * 128:(lc4 + 1) * 128],
                                 bsk8, half * 4 + lc4, rs,
                                 stop=(lc4 == 3))
                    sg = sgtl[:, 0, half * 512:(half + 1) * 512]
                    # (s+b) psum -> SBUF bf16; mostly DVE (ACT is exp-bound)
                    stc_act = STC_MOD > 0 and \
                        (h * 16 + rs * 2 + half) % STC_MOD == 0
                    if use_m:
                        if stc_act:
                            nc.scalar.activation(
                                sg, st,
                                mybir.ActivationFunctionType.Identity,
                                bias=m_sb[:, rs, 1:2])
                        else:
                            nc.vector.tensor_scalar_add(
                                sg, st, m_sb[:, rs, 1:2])
                    elif stc_act:
                        nc.scalar.activation(sg, st, COPY)
                    else:
                        nc.vector.tensor_copy(sg, st)
                    nc.gpsimd.tensor_mul(
                        sgtl[:, 1, half * 512:(half + 1) * 512], sg,
                        smT_sb[:, rs, half * 512:(half + 1) * 512])
                pgl = pglp.tile([128, 2, S], BF16, tag="pgl")
                if use_m:
                    nc.scalar.activation(pgl[:, 0, :], sgtl[:, 0, :],
                                         EXP, scale=0.125)
                    nc.scalar.activation(pgl[:, 1, :], sgtl[:, 1, :],
                                         EXP, scale=0.125,
                                         bias=m_sb[:, rs, 0:1])
                else:
                    nc.scalar.activation(pgl, sgtl, EXP, scale=0.125)
                return pgl

            def emit_S(h, front0=None):
                vv = v4t_sb.rearrange("p rs (h d) -> p rs h d", d=HD)[:, :, h, :]
                pcv = ps_pc.tile([128, LTN, 2, HD], F32, tag="pcv")
                pcs = ps_pc.tile([128, LTN, 2, 1], F32, tag="pcs")
                next_front = None
                for rs in range(8):
                    pgl = front0 if (rs == 0 and front0 is not None) \
                        else s_front(h, rs)
                    if rs == 7 and h + 1 < HPC:
                        # cross-head pipelining: next head's first front runs
                        # its exp while this head's last ctx waits
                        next_front = s_front(h + 1, 0)
                    # pcv spans 2 psum banks (lt 0-3 / 4-7); pcs one more.
                    # start_tensor_calc arms zero-on-write per 2KB bank, so
                    # exactly one start per bank and one stop at the bank's
                    # last matmul (PE executes in emission order).
                    for br in range(2):
                        for lt in range(LTN):
                            lw = pgl[:, br, lt * 128:(lt + 1) * 128]
                            nc.tensor.matmul(
                                pcv[:, lt, br, :], lhsT=lw,
                                rhs=vv[:, rs, :],
                                start=(rs == 0 and br == 0 and lt % 4 == 0),
                                stop=(rs == 7 and br == 1 and lt % 4 == 3),
                                skip_group_check=True,
                            )
                            nc.tensor.matmul(
                                pcs[:, lt, br, :], lhsT=lw,
                                rhs=ones1,
                                start=(rs == 0 and br == 0 and lt == 0),
                                stop=(rs == 7 and br == 1 and lt == 7),
                                skip_group_check=True,
                            )
                    if h == 0 and rs == 1:
                        emit_G(2, ps_s, tag="st", on_act=False)
                        # k2 re-based to partitions 0-63 (engines cannot
                        # cross partitions)
                        nc.sync.dma_start(out=k2lo_sb, in_=qkT_sb[64:128, 2, :])
                    if h + 1 < HPC and rs < 4:
                        if rs == 0:
                            r_begin(h + 1)
                        r_unit(h + 1, 2 * rs)
                        r_unit(h + 1, 2 * rs + 1)
                return pcv, pcs, next_front

            def emit_F(h, pcv, pcs):
                pcb = finp.tile([128, LTN, 2, HD], F32, tag="pcb")
                rsum = finp.tile([128, LTN, 2], F32, tag="rsum")
                w = finp.tile([128, LTN, 2], F32, tag="w")
                tmp = finp.tile([128, LTN, HD], F32, tag="tmp")
                tmp2 = finp.tile([128, LTN, HD], F32, tag="tmp2")
                ov = osb.rearrange("p lc (h d) -> p lc h d", d=HD)[:, :, h, :]
                for q in range(2):
                    a, b = q * 4, q * 4 + 4
                    nc.vector.tensor_copy(pcb[:, a:b], pcv[:, a:b])
                    nc.vector.reciprocal(rsum[:, a:b], pcs[:, a:b, :, 0])
                    nc.gpsimd.tensor_mul(w[:, a:b], rsum[:, a:b],
                                         selw_sb[:, a:b])
                    nc.gpsimd.tensor_mul(tmp[:, a:b], pcb[:, a:b, 1, :],
                                         _bcast(w[:, a:b, 1], HD, 2))
                    nc.gpsimd.tensor_mul(tmp2[:, a:b], pcb[:, a:b, 0, :],
                                         _bcast(w[:, a:b, 0], HD, 2))
                    nc.gpsimd.tensor_add(ov[:, a:b], tmp[:, a:b],
                                         tmp2[:, a:b])
                    outv = outp.rearrange("(lc p) c -> p lc c", p=128)
                    nc.sync.dma_start(
                        out=outv[:, a:b, h * HD:(h + 1) * HD],
                        in_=osb[:, a:b, h * HD:(h + 1) * HD])

            front = None
            for h in range(HPC):
                pcv, pcs, front = emit_S(h, front)
                emit_F(h, pcv, pcs)

    nc.compile()
    return nc


def _get_program(n_cc, use_m):
    key = (n_cc, use_m)
    if key not in _programs:
        _programs[key] = build_program(n_cc, use_m)
    return _programs[key]


def kernel(hidden_states, attention_mask, scaled_attention_mask, selector_outputs,
           Wq, bq, Wk, bk, Wv, bv, dist_emb):
    hidden_states = np.asarray(hidden_states, np.float32)
    attention_mask = np.asarray(attention_mask, np.float32)
    scaled_attention_mask = np.asarray(scaled_attention_mask, np.float32)
    selector_outputs = np.asarray(selector_outputs, np.float32)
    Wq, Wk, Wv = (np.asarray(x, np.float32) for x in (Wq, Wk, Wv))
    bq, bk, bv = (np.asarray(x, np.float32) for x in (bq, bk, bv))
    dist_emb = np.asarray(dist_emb, np.float32)

    use_bias = bool(np.any(bq) or np.any(bk) or np.any(bv))
    use_m = bool(np.any(attention_mask))
    n_cc = 7 if use_bias else 6
    CH = n_cc * 128
    nc = _get_program(n_cc, use_m)

    smT = np.ascontiguousarray(scaled_attention_mask[0, 0].T).astype(ml_dtypes.float8_e4m3)
    e_rev_t = dist_emb[::-1].T.astype(NPBF16)
    embT2 = np.ascontiguousarray(np.concatenate([e_rev_t, e_rev_t], axis=0))

    in_maps = []
    for core in range(NCORES):
        b = core // 4
        k4 = core % 4
        heads = [3 * k4, 3 * k4 + 1, 3 * k4 + 2]

        hidT = hidden_states[b].T  # [768, S]
        if use_bias:
            hidT = np.concatenate(
                [hidT, np.ones((1, S), np.float32),
                 np.zeros((CH - HID - 1, S), np.float32)], axis=0)
        hidT_bf = np.ascontiguousarray(hidT).astype(NPBF16)

        def wcols(W, bvec, h):
            c = W[:, h * HD:(h + 1) * HD]
            if use_bias:
                c = np.concatenate(
                    [c, bvec[None, h * HD:(h + 1) * HD],
                     np.zeros((CH - HID - 1, HD), np.float32)], axis=0)
            return c

        q0, q1, q2 = (wcols(Wq, bq, h) for h in heads)
        k0, k1, k2 = (wcols(Wk, bk, h) for h in heads)
        wg_np = np.stack([
            np.concatenate([q0, q1], axis=1),
            np.concatenate([k0, k1], axis=1),
            np.concatenate([q2, k2], axis=1),
        ]).astype(NPBF16)
        wv_np = np.concatenate(
            [wcols(Wv, bv, h) for h in heads], axis=1).astype(NPBF16)

        sel = selector_outputs[b, 0, :, 0]
        selw_np = np.stack([1.0 - sel, sel], axis=1).astype(np.float32)

        m = {
            "hidT": hidT_bf,
            "wg": wg_np,
            "wv": np.ascontiguousarray(wv_np),
            "embT2": embT2,
            "smT": smT,
            "selw": np.ascontiguousarray(selw_np),
        }
        if use_m:
            mv = attention_mask[b, 0, 0]
            m["mvec"] = np.ascontiguousarray(
                np.stack([mv, 8.0 * mv], axis=1).astype(np.float32))
        in_maps.append(m)

    res = run_bass_kernel_spmd(nc, in_maps, list(range(NCORES)))

    out = np.empty((B, S, HID), np.float32)
    for core in range(NCORES):
        b = core // 4
        k4 = core % 4
        out[b, :, 192 * k4:192 * (k4 + 1)] = res.results[core]["out"]
    return out


# revision 39
# speedup vs baseline: 1.3654x; 1.0022x over previous
"""Bass/Trainium2 kernel for nn_BayesianBertSelfAttention (B=2,S=1024,HID=768,NH=12,HD=64).

Sharding: 24 (batch, head) pairs over 8 cores -> core c handles batch c//4,
heads {3k, 3k+1, 3k+2} with k = c%4.

Per-core device algorithm (transposed-scores layout, scoresT[r, l]):
  phase P: q/k projections as 3 column-packed matmul groups ([q0|q1], [k0|k1],
           [q2|k2]); k2 re-based to partitions 0-63 via an SBUF->SBUF DMA.
           v projections in [r, d] layout (bf16).
  phase R (interleaved into S): relative-position band R'[l, c] = q . E_rev
           per 128-row l-tile (bf16 matmul), copied fp8 to a DRAM scratch with
           row pitch 1152; skewed fp8 read-back (Music-Transformer skew as a
           strided DRAM access pattern) gives bias[l, r] tiles in SBUF.
  phase S: per (head, r-chunk, l-half): the fp8 bias tiles are transposed AND
           added into the f32 score psum by regular fp8 matmuls with an
           identity rhs (out += bias_tile^T @ I), paired with a zeros slot so
           fp8 DoubleRow mode runs them at 2 rows/cycle. One psum->SBUF copy
           (ACT or DVE) materializes (s+b) bf16; GPSIMD multiplies by SM^T for
           the local branch; one fused ACT exp(scale=1/8) per r-chunk covers
           the [128, 2*1024] global|local pair. Context accumulated directly
           in [l, d] orientation: lhsT = probs chunk [r, l], rhs = v [r, d] ->
           psum [l, d] (plus a ones-column matmul for softmax row sums).
  phase F: per head, no transposes: DVE copies ctx psum to SBUF + reciprocal
           of row sums; GPSIMD blends the two branches with selector weights;
           per-head strided output DMA.

Host: packs weights/binds layouts, converts to bf16, reassembles [2,1024,768].
"""

import os
import sys

sys.path.insert(0, "/opt/trn_rl_repo")

import numpy as np
import ml_dtypes
from contextlib import ExitStack

import concourse.bass as bass
import concourse.bacc as bacc
import concourse.tile as tile
from concourse import mybir
from concourse.bass_utils import run_bass_kernel_spmd
from concourse.masks import make_identity

B, S, HID, NH, HD = 2, 1024, 768, 12, 64
MAXP = 1024
NCORES = 8
HPC = 3            # heads per core
LTN = S // 128     # 8 l-tiles
BAND = 1151        # skew band width per 128-row l-tile
PITCH = 1152       # skew scratch row pitch
NE = 2 * MAXP - 1  # 2047

BF16 = mybir.dt.bfloat16
F32 = mybir.dt.float32
FP8 = mybir.dt.float8e4
COPY = mybir.ActivationFunctionType.Copy
EXP = mybir.ActivationFunctionType.Exp
ADD = mybir.AluOpType.add

NPBF16 = ml_dtypes.bfloat16

_programs = {}

# engine-schedule knobs (sweepable via env for tuning)
RT_H0 = os.environ.get("K_RT_H0", "act2")     # head-0 rt copies: alt|act2
RT_HS = os.environ.get("K_RT_HS", "h1heavy")  # later heads: dve|pi0|pi2|pi02|h1heavy|h1all4
STC_MOD = int(os.environ.get("K_STC_MOD", "0"))  # 1 in N stc halves on ACT (0=never)
R_SPREAD = os.environ.get("K_R_SPREAD", "front4")  # front4|all8
HOLD_MS = float(os.environ.get("K_HOLD_MS", "0"))  # bulk-load dispatch hold


def _bcast(ap, dim_count, insert_at):
    """Insert a step-0 broadcast dim of size dim_count at free-dim position."""
    new = list(ap.ap)
    new.insert(insert_at, [0, dim_count])
    return bass.AP(tensor=ap.tensor, offset=ap.offset, ap=new)


def _dram_ap(ap, offset, dims):
    """Raw DRAM access pattern on ap's tensor (element units)."""
    return bass.AP(tensor=ap.tensor, offset=offset, ap=[list(d) for d in dims])


def build_program(n_cc=6, use_m=False):
    """n_cc: number of 128-row contraction chunks for projections (6 normally,
    7 when biases are nonzero and folded in as an extra ones row)."""
    nc = bacc.Bacc(None)
    CH = n_cc * 128

    hidT = nc.dram_tensor("hidT", [CH, S], BF16, kind="ExternalInput")
    wg = nc.dram_tensor("wg", [3, CH, 128], BF16, kind="ExternalInput")
    wv = nc.dram_tensor("wv", [CH, HPC * HD], BF16, kind="ExternalInput")
    embT2 = nc.dram_tensor("embT2", [128, NE], BF16, kind="ExternalInput")
    smT = nc.dram_tensor("smT", [S, S], FP8, kind="ExternalInput")
    selw = nc.dram_tensor("selw", [S, 2], F32, kind="ExternalInput")
    if use_m:
        mvec = nc.dram_tensor("mvec", [S, 2], F32, kind="ExternalInput")  # [m, 8m]
    outp = nc.dram_tensor("out", [S, HPC * HD], F32, kind="ExternalOutput")
    skews = [nc.dram_tensor(f"skew{h}", [LTN * 128 * PITCH], FP8)
             for h in range(HPC)]

    # (q_h, k_h) G-group/partition offsets: G0=[q0|q1], G1=[k0|k1], G2=[q2|k2]
    QG = [(0, 0), (0, 64), (2, 0)]
    KG = [(1, 0), (1, 64), None]  # head 2 k comes from k2lo_sb

    with tile.TileContext(nc) as tc, ExitStack() as ctx:
        singles = ctx.enter_context(tc.tile_pool(name="singles", bufs=1))

        hid_sb = singles.tile([128, n_cc, S], BF16)
        wg_sb = singles.tile([128, 3, n_cc, 128], BF16)
        wv_sb = singles.tile([128, n_cc, HPC * HD], BF16)
        emb_sb = singles.tile([128, NE], BF16)
        smT_sb = singles.tile([128, 8, S], FP8)
        selw_sb = singles.tile([128, 8, 2], F32)
        qkT_sb = singles.tile([128, 3, S], BF16)
        k2lo_sb = singles.tile([64, S], BF16)
        v4t_sb = singles.tile([128, 8, HPC * HD], BF16)
        ones1 = singles.tile([128, 1], BF16)
        id8z = singles.tile([128, 2, 128], FP8)  # [identity | zeros]
        osb = singles.tile([128, LTN, HPC * HD], F32)

        hid_v = hidT.rearrange("(cc p) l -> p cc l", p=128)
        wg_v = wg.rearrange("g (cc p) d -> p g cc d", p=128)
        # priority order on the serial DMA resource: weights for G0, hid,
        # emb (phase R), wg1/wv; the first smT quarter and wg2/smT-rest are
        # queued around head 0's skew round trip so S(0) can start early.
        nc.sync.dma_start(out=wg_sb[:, 0], in_=wg_v[:, 0])
        cch = n_cc // 2
        nc.sync.dma_start(out=hid_sb[:, 0:cch], in_=hid_v[:, 0:cch])
        nc.sync.dma_start(out=hid_sb[:, cch:n_cc], in_=hid_v[:, cch:n_cc])
        nc.sync.dma_start(out=emb_sb, in_=embT2[:, :])
        nc.sync.dma_start(out=wg_sb[:, 1], in_=wg_v[:, 1])
        nc.sync.dma_start(out=wv_sb, in_=wv.rearrange("(cc p) d -> p cc d", p=128))
        if use_m:
            m_sb = singles.tile([128, 8, 2], F32)
            nc.sync.dma_start(out=m_sb, in_=mvec.rearrange("(rs p) w -> p rs w", p=128))

        nc.vector.memset(ones1, 1.0)
        nc.vector.memset(id8z[:, 1, :], 0.0)
        make_identity(nc, id8z[:, 0, :])

        smT_v = smT.rearrange("(rs p) l -> p rs l", p=128)

        # R-phase psum pool lives across both P and S phases.
        ps_rb = ctx.enter_context(tc.tile_pool(name="ps_rb", bufs=3, space="PSUM"))
        rtp = ctx.enter_context(tc.tile_pool(name="rtp", bufs=3))
        bskp = ctx.enter_context(tc.tile_pool(name="bskp", bufs=3))
        sgtlp = ctx.enter_context(tc.tile_pool(name="sgtlp", bufs=3))
        pglp = ctx.enter_context(tc.tile_pool(name="pglp", bufs=3))
        finp = ctx.enter_context(tc.tile_pool(name="finp", bufs=2))

        rstate = {}
        ZS = LTN  # zeros slot index in bsk8 (for fp8 DoubleRow pairing)

        def r_begin(h):
            rt = rtp.tile([128, LTN, PITCH], FP8, tag="rt")
            bsk8 = bskp.tile([128, LTN + 1, S], FP8, tag="bsk")
            nc.gpsimd.memset(bsk8[:, ZS, :], 0.0)
            rstate[h] = (rt, bsk8)

        def r_unit(h, lt):
            """Band matmuls + psum->fp8 copies for one l-tile of head h.
            Head 0 runs in the P window where ACT is idle; later heads'
            copies go to DVE to keep ACT free for the exps."""
            rt, bsk8 = rstate[h]
            qg, qpo = QG[h]
            qb = qkT_sb[qpo:qpo + 64, qg, :]
            qbl = qb[:, lt * 128:(lt + 1) * 128]
            e0 = 896 - lt * 128
            pieces = ((0, 512), (512, 1024), (1024, BAND))
            for pi, (a, b) in enumerate(pieces):
                pr = ps_rb.tile([128, 512], F32, tag="rb")
                nc.tensor.matmul(
                    pr[:, 0:b - a], lhsT=qbl,
                    rhs=emb_sb[qpo:qpo + 64, e0 + a:e0 + b],
                    start=True, stop=True,
                )
                if h == 0:
                    if RT_H0 == "alt":
                        on_act = (lt + pi) % 2 == 0
                    else:
                        on_act = pi == 2 or (lt + pi) % 2 == 0
                elif RT_HS == "dve":
                    on_act = False
                elif RT_HS == "pi0":
                    on_act = pi == 0
                elif RT_HS == "pi2":
                    on_act = pi == 2
                elif RT_HS == "h1heavy":
                    on_act = pi in (0, 2) if h == 1 else pi == 0
                elif RT_HS == "h1all4":
                    if h == 1:
                        on_act = True if lt < 4 else pi in (0, 2)
                    else:
                        on_act = pi == 0
                else:
                    on_act = pi in (0, 2)
                if on_act:
                    nc.scalar.activation(rt[:, lt, a:b], pr[:, 0:b - a], COPY)
                else:
                    nc.vector.tensor_copy(rt[:, lt, a:b], pr[:, 0:b - a])
            if lt % 2 == 1:
                lt0 = lt - 1
                # the skew round trip gates the next S phase: dispatch these
                # ahead of bulk loads (smT/wg2) on the serial DMA resource
                with tc.high_priority():
                    # skew write: rows (lt, l_loc), cols [0, BAND), pitch PITCH
                    wview = _dram_ap(
                        skews[h][:], lt0 * 128 * PITCH,
                        [[PITCH, 128], [128 * PITCH, 2], [1, BAND]])
                    nc.sync.dma_start(out=wview, in_=rt[:, lt0:lt0 + 2, 0:BAND])
                    # skewed read-back: bias[l, r] per l-tile
                    rview = _dram_ap(
                        skews[h][:], lt0 * 128 * PITCH + 127,
                        [[BAND, 128], [128 * PITCH, 2], [1, S]])
                    nc.sync.dma_start(out=bsk8[:, lt0:lt0 + 2, :], in_=rview)

        def emit_G(g, pool, tag="pt", on_act=True):
            for n in range(2):
                pt = pool.tile([128, 512], F32, tag=tag)
                for cc in range(n_cc):
                    nc.tensor.matmul(
                        pt,
                        lhsT=wg_sb[:, g, cc, :],
                        rhs=hid_sb[:, cc, n * 512:(n + 1) * 512],
                        start=(cc == 0), stop=(cc == n_cc - 1),
                    )
                if on_act:
                    nc.scalar.activation(qkT_sb[:, g, n * 512:(n + 1) * 512],
                                         pt, COPY)
                else:
                    nc.vector.tensor_copy(qkT_sb[:, g, n * 512:(n + 1) * 512],
                                          pt)

        # ---- phase P: projections (G2 deferred into S(0): wg2 loads late) ----
        with tc.tile_pool(name="ps_pt", bufs=2, space="PSUM") as ps_pt:
            emit_G(0, ps_pt)
            nc.sync.dma_start(out=smT_sb[:, 0:2], in_=smT_v[:, 0:2])
            for lt in range(LTN):
                if lt == 0:
                    r_begin(0)
                r_unit(0, lt)
            emit_G(1, ps_pt, on_act=False)
            with tc.tile_wait_until(HOLD_MS):
                nc.sync.dma_start(out=smT_sb[:, 2:8], in_=smT_v[:, 2:8])
            for rs in range(8):
                pv = ps_pt.tile([128, HPC * HD], F32, tag="pt")
                for cc in range(n_cc):
                    nc.tensor.matmul(
                        pv,
                        lhsT=hid_sb[:, cc, rs * 128:(rs + 1) * 128],
                        rhs=wv_sb[:, cc, :],
                        start=(cc == 0), stop=(cc == n_cc - 1),
                    )
                nc.vector.tensor_copy(v4t_sb[:, rs, :], pv)
            with tc.tile_wait_until(HOLD_MS):
                nc.sync.dma_start(out=wg_sb[:, 2], in_=wg_v[:, 2])
                nc.sync.dma_start(out=selw_sb,
                                  in_=selw.rearrange("(lc p) w -> p lc w",
                                                     p=128))

        # ---- phase S + F, with R(h+1) interleaved into S(h) ----
        with tc.tile_pool(name="ps_s", bufs=2, space="PSUM") as ps_s, \
             tc.tile_pool(name="ps_pc", bufs=1, space="PSUM") as ps_pc:

            id8z_pair = id8z[:, :, :]

            def bias_acc(st_slice, bsk8, lt, rs, stop):
                """st_slice += bias_tile^T via a regular fp8 matmul with an
                identity rhs; the zeros slot pairs the lhsT/rhs so fp8
                DoubleRow mode runs 2 rows/cycle."""
                base = bsk8[:, lt, rs * 128:(rs + 1) * 128]
                lhsT = bass.AP(
                    tensor=base.tensor, offset=base.offset,
                    ap=[list(base.ap[0]), [(ZS - lt) * S, 2]]
                       + [list(d) for d in base.ap[1:]])
                nc.tensor.matmul(
                    st_slice, lhsT=lhsT, rhs=id8z_pair,
                    start=False, stop=stop,
                    perf_mode=mybir.MatmulPerfMode.DoubleRow,
                )

            def s_front(h, rs):
                """scores + bias + (s+b) staging + local mul + fused exp ->
                pgl for one r-chunk."""
                _, bsk8 = rstate[h]
                qg, qpo = QG[h]
                qf = qkT_sb[qpo:qpo + 64, qg, :]
                if h < 2:
                    kg, kpo = KG[h]
                    kf = qkT_sb[kpo:kpo + 64, kg, :]
                else:
                    kf = k2lo_sb[:, :]
                sgtl = sgtlp.tile([128, 2, S], BF16, tag="sgtl")
                for half in range(2):
                    st = ps_s.tile([128, 512], F32, tag="st")
                    nc.tensor.matmul(
                        st,
                        lhsT=kf[:, rs * 128:(rs + 1) * 128],
                        rhs=qf[:, half * 512:(half + 1) * 512],
                        start=True, stop=False,
                    )
                    for lc4 in range(4):
                        bias_acc(st[:, lc4 * 128:(lc4 + 1) * 128],
                                 bsk8, half * 4 + lc4, rs,
                                 stop=(lc4 == 3))
                    sg = sgtl[:, 0, half * 512:(half + 1) * 512]
                    # (s+b) psum -> SBUF bf16; mostly DVE (ACT is exp-bound)
                    stc_act = STC_MOD > 0 and \
                        (h * 16 + rs * 2 + half) % STC_MOD == 0
                    if use_m:
                        if stc_act:
                            nc.scalar.activation(
                                sg, st,
                                mybir.ActivationFunctionType.Identity,
                                bias=m_sb[:, rs, 1:2])
                        else:
                            nc.vector.tensor_scalar_add(
                                sg, st, m_sb[:, rs, 1:2])
                    elif stc_act:
                        nc.scalar.activation(sg, st, COPY)
                    else:
                        nc.vector.tensor_copy(sg, st)
                    nc.gpsimd.tensor_mul(
                        sgtl[:, 1, half * 512:(half + 1) * 512], sg,
                        smT_sb[:, rs, half * 512:(half + 1) * 512])
                pgl = pglp.tile([128, 2, S], BF16, tag="pgl")
                if use_m:
                    nc.scalar.activation(pgl[:, 0, :], sgtl[:, 0, :],
                                         EXP, scale=0.125)
                    nc.scalar.activation(pgl[:, 1, :], sgtl[:, 1, :],
                                         EXP, scale=0.125,
                                         bias=m_sb[:, rs, 0:1])
                else:
                    nc.scalar.activation(pgl, sgtl, EXP, scale=0.125)
                return pgl

            def emit_S(h, front0=None):
                vv = v4t_sb.rearrange("p rs (h d) -> p rs h d", d=HD)[:, :, h, :]
                pcv = ps_pc.tile([128, LTN, 2, HD], F32, tag="pcv")
                pcs = ps_pc.tile([128, LTN, 2, 1], F32, tag="pcs")
                next_front = None
                for rs in range(8):
                    if front0 is not None and rs < len(front0):
                        pgl = front0[rs]
                    else:
                        pgl = s_front(h, rs)
                    if rs == 7 and h + 1 < HPC:
                        # cross-head pipelining: next head's first front runs
                        # its exp while this head's last ctx waits
                        next_front = [s_front(h + 1, 0)]
                    # pcv spans 2 psum banks (lt 0-3 / 4-7); pcs one more.
                    # start_tensor_calc arms zero-on-write per 2KB bank, so
                    # exactly one start per bank and one stop at the bank's
                    # last matmul (PE executes in emission order).
                    for br in range(2):
                        for lt in range(LTN):
                            lw = pgl[:, br, lt * 128:(lt + 1) * 128]
                            nc.tensor.matmul(
                                pcv[:, lt, br, :], lhsT=lw,
                                rhs=vv[:, rs, :],
                                start=(rs == 0 and br == 0 and lt % 4 == 0),
                                stop=(rs == 7 and br == 1 and lt % 4 == 3),
                                skip_group_check=True,
                            )
                            nc.tensor.matmul(
                                pcs[:, lt, br, :], lhsT=lw,
                                rhs=ones1,
                                start=(rs == 0 and br == 0 and lt == 0),
                                stop=(rs == 7 and br == 1 and lt == 7),
                                skip_group_check=True,
                            )
                    if h == 0 and rs == 1:
                        emit_G(2, ps_s, tag="st", on_act=False)
                        # k2 re-based to partitions 0-63 (engines cannot
                        # cross partitions)
                        nc.sync.dma_start(out=k2lo_sb, in_=qkT_sb[64:128, 2, :])
                    if h + 1 < HPC and rs < 4:
                        if rs == 0:
                            r_begin(h + 1)
                        r_unit(h + 1, 2 * rs)
                        r_unit(h + 1, 2 * rs + 1)
                return pcv, pcs, next_front

            def emit_F(h, pcv, pcs):
                pcb = finp.tile([128, LTN, 2, HD], F32, tag="pcb")
                rsum = finp.tile([128, LTN, 2], F32, tag="rsum")
                w = finp.tile([128, LTN, 2], F32, tag="w")
                tmp = finp.tile([128, LTN, HD], F32, tag="tmp")
                tmp2 = finp.tile([128, LTN, HD], F32, tag="tmp2")
                ov = osb.rearrange("p lc (h d) -> p lc h d", d=HD)[:, :, h, :]
                for q in range(2):
                    a, b = q * 4, q * 4 + 4
                    nc.vector.tensor_copy(pcb[:, a:b], pcv[:, a:b])
                    nc.vector.reciprocal(rsum[:, a:b], pcs[:, a:b, :, 0])
                    nc.gpsimd.tensor_mul(w[:, a:b], rsum[:, a:b],
                                         selw_sb[:, a:b])
                    nc.gpsimd.tensor_mul(tmp[:, a:b], pcb[:, a:b, 1, :],
                                         _bcast(w[:, a:b, 1], HD, 2))
                    nc.gpsimd.tensor_mul(tmp2[:, a:b], pcb[:, a:b, 0, :],
                                         _bcast(w[:, a:b, 0], HD, 2))
                    nc.gpsimd.tensor_add(ov[:, a:b], tmp[:, a:b],
                                         tmp2[:, a:b])
                    outv = outp.rearrange("(lc p) c -> p lc c", p=128)
                    nc.sync.dma_start(
                        out=outv[:, a:b, h * HD:(h + 1) * HD],
                        in_=osb[:, a:b, h * HD:(h + 1) * HD])

            front = None
            for h in range(HPC):
                pcv, pcs, front = emit_S(h, front)
                emit_F(h, pcv, pcs)

    nc.compile()
    return nc


def _get_program(n_cc, use_m):
    key = (n_cc, use_m)
    if key not in _programs:
        _programs[key] = build_program(n_cc, use_m)
    return _programs[key]


def kernel(hidden_states, attention_mask, scaled_attention_mask, selector_outputs,
           Wq, bq, Wk, bk, Wv, bv, dist_emb):
    hidden_states = np.asarray(hidden_states, np.float32)
    attention_mask = np.asarray(attention_mask, np.float32)
    scaled_attention_mask = np.asarray(scaled_attention_mask, np.float32)
    selector_outputs = np.asarray(selector_outputs, np.float32)
    Wq, Wk, Wv = (np.asarray(x, np.float32) for x in (Wq, Wk, Wv))
    bq, bk, bv = (np.asarray(x, np.float32) for x in (bq, bk, bv))
    dist_emb = np.asarray(dist_emb, np.float32)

    use_bias = bool(np.any(bq) or np.any(bk) or np.any(bv))
    use_m = bool(np.any(attention_mask))
    n_cc = 7 if use_bias else 6
    CH = n_cc * 128
    nc = _get_program(n_cc, use_m)

    smT = np.ascontiguousarray(scaled_attention_mask[0, 0].T).astype(ml_dtypes.float8_e4m3)
    e_rev_t = dist_emb[::-1].T.astype(NPBF16)
    embT2 = np.ascontiguousarray(np.concatenate([e_rev_t, e_rev_t], axis=0))

    in_maps = []
    for core in range(NCORES):
        b = core // 4
        k4 = core % 4
        heads = [3 * k4, 3 * k4 + 1, 3 * k4 + 2]

        hidT = hidden_states[b].T  # [768, S]
        if use_bias:
            hidT = np.concatenate(
                [hidT, np.ones((1, S), np.float32),
                 np.zeros((CH - HID - 1, S), np.float32)], axis=0)
        hidT_bf = np.ascontiguousarray(hidT).astype(NPBF16)

        def wcols(W, bvec, h):
            c = W[:, h * HD:(h + 1) * HD]
            if use_bias:
                c = np.concatenate(
                    [c, bvec[None, h * HD:(h + 1) * HD],
                     np.zeros((CH - HID - 1, HD), np.float32)], axis=0)
            return c

        q0, q1, q2 = (wcols(Wq, bq, h) for h in heads)
        k0, k1, k2 = (wcols(Wk, bk, h) for h in heads)
        wg_np = np.stack([
            np.concatenate([q0, q1], axis=1),
            np.concatenate([k0, k1], axis=1),
            np.concatenate([q2, k2], axis=1),
        ]).astype(NPBF16)
        wv_np = np.concatenate(
            [wcols(Wv, bv, h) for h in heads], axis=1).astype(NPBF16)

        sel = selector_outputs[b, 0, :, 0]
        selw_np = np.stack([1.0 - sel, sel], axis=1).astype(np.float32)

        m = {
            "hidT": hidT_bf,
            "wg": wg_np,
            "wv": np.ascontiguousarray(wv_np),
            "embT2": embT2,
            "smT": smT,
            "selw": np.ascontiguousarray(selw_np),
        }
        if use_m:
            mv = attention_mask[b, 0, 0]
            m["mvec"] = np.ascontiguousarray(
                np.stack([mv, 8.0 * mv], axis=1).astype(np.float32))
        in_maps.append(m)

    res = run_bass_kernel_spmd(nc, in_maps, list(range(NCORES)))

    out = np.empty((B, S, HID), np.float32)
    for core in range(NCORES):
        b = core // 4
        k4 = core % 4
        out[b, :, 192 * k4:192 * (k4 + 1)] = res.results[core]["out"]
    return out


# revision 41
# speedup vs baseline: 1.3837x; 1.0134x over previous
"""Bass/Trainium2 kernel for nn_BayesianBertSelfAttention (B=2,S=1024,HID=768,NH=12,HD=64).

Sharding: 24 (batch, head) pairs over 8 cores -> core c handles batch c//4,
heads {3k, 3k+1, 3k+2} with k = c%4.

Per-core device algorithm (transposed-scores layout, scoresT[r, l]):
  phase P: q/k projections as 3 column-packed matmul groups ([q0|q1], [k0|k1],
           [q2|k2]); k2 re-based to partitions 0-63 via an SBUF->SBUF DMA.
           v projections in [r, d] layout (bf16).
  phase R (interleaved into S): relative-position band R'[l, c] = q . E_rev
           per 128-row l-tile (bf16 matmul), copied fp8 to a DRAM scratch with
           row pitch 1152; skewed fp8 read-back (Music-Transformer skew as a
           strided DRAM access pattern) gives bias[l, r] tiles in SBUF.
  phase S: per (head, r-chunk, l-half): the fp8 bias tiles are transposed AND
           added into the f32 score psum by regular fp8 matmuls with an
           identity rhs (out += bias_tile^T @ I), paired with a zeros slot so
           fp8 DoubleRow mode runs them at 2 rows/cycle. One psum->SBUF copy
           (ACT or DVE) materializes (s+b) bf16; GPSIMD multiplies by SM^T for
           the local branch; one fused ACT exp(scale=1/8) per r-chunk covers
           the [128, 2*1024] global|local pair. Context accumulated directly
           in [l, d] orientation: lhsT = probs chunk [r, l], rhs = v [r, d] ->
           psum [l, d] (plus a ones-column matmul for softmax row sums).
  phase F: per head, no transposes: DVE copies ctx psum to SBUF + reciprocal
           of row sums; GPSIMD blends the two branches with selector weights;
           per-head strided output DMA.

Host: packs weights/binds layouts, converts to bf16, reassembles [2,1024,768].
"""

import os
import sys

sys.path.insert(0, "/opt/trn_rl_repo")

import numpy as np
import ml_dtypes
from contextlib import ExitStack

import concourse.bass as bass
import concourse.bacc as bacc
import concourse.tile as tile
from concourse import mybir
from concourse.bass_utils import run_bass_kernel_spmd
from concourse.masks import make_identity

B, S, HID, NH, HD = 2, 1024, 768, 12, 64
MAXP = 1024
NCORES = 8
HPC = 3            # heads per core
LTN = S // 128     # 8 l-tiles
BAND = 1151        # skew band width per 128-row l-tile
PITCH = 1152       # skew scratch row pitch
NE = 2 * MAXP - 1  # 2047

BF16 = mybir.dt.bfloat16
F32 = mybir.dt.float32
FP8 = mybir.dt.float8e4
COPY = mybir.ActivationFunctionType.Copy
EXP = mybir.ActivationFunctionType.Exp
ADD = mybir.AluOpType.add

NPBF16 = ml_dtypes.bfloat16

_programs = {}

# engine-schedule knobs (sweepable via env for tuning)
RT_H0 = os.environ.get("K_RT_H0", "act2")     # head-0 rt copies: alt|act2
RT_HS = os.environ.get("K_RT_HS", "h1heavy")  # later heads: dve|pi0|pi2|pi02|h1heavy|h1all4
STC_MOD = int(os.environ.get("K_STC_MOD", "0"))  # 1 in N stc halves on ACT (0=never)
R_SPREAD = os.environ.get("K_R_SPREAD", "front4")  # front4|all8
HOLD_MS = float(os.environ.get("K_HOLD_MS", "0"))  # bulk-load dispatch hold
N_WARM = int(os.environ.get("K_N_WARM", "20"))  # PE p-state warm-up matmuls


def _bcast(ap, dim_count, insert_at):
    """Insert a step-0 broadcast dim of size dim_count at free-dim position."""
    new = list(ap.ap)
    new.insert(insert_at, [0, dim_count])
    return bass.AP(tensor=ap.tensor, offset=ap.offset, ap=new)


def _dram_ap(ap, offset, dims):
    """Raw DRAM access pattern on ap's tensor (element units)."""
    return bass.AP(tensor=ap.tensor, offset=offset, ap=[list(d) for d in dims])


def build_program(n_cc=6, use_m=False):
    """n_cc: number of 128-row contraction chunks for projections (6 normally,
    7 when biases are nonzero and folded in as an extra ones row)."""
    nc = bacc.Bacc(None)
    CH = n_cc * 128

    hidT = nc.dram_tensor("hidT", [CH, S], BF16, kind="ExternalInput")
    wg = nc.dram_tensor("wg", [3, CH, 128], BF16, kind="ExternalInput")
    wv = nc.dram_tensor("wv", [CH, HPC * HD], BF16, kind="ExternalInput")
    embT2 = nc.dram_tensor("embT2", [128, NE], BF16, kind="ExternalInput")
    smT = nc.dram_tensor("smT", [S, S], FP8, kind="ExternalInput")
    selw = nc.dram_tensor("selw", [S, 2], F32, kind="ExternalInput")
    if use_m:
        mvec = nc.dram_tensor("mvec", [S, 2], F32, kind="ExternalInput")  # [m, 8m]
    outp = nc.dram_tensor("out", [S, HPC * HD], F32, kind="ExternalOutput")
    skews = [nc.dram_tensor(f"skew{h}", [LTN * 128 * PITCH], FP8)
             for h in range(HPC)]

    # (q_h, k_h) G-group/partition offsets: G0=[q0|q1], G1=[k0|k1], G2=[q2|k2]
    QG = [(0, 0), (0, 64), (2, 0)]
    KG = [(1, 0), (1, 64), None]  # head 2 k comes from k2lo_sb

    with tile.TileContext(nc) as tc, ExitStack() as ctx:
        singles = ctx.enter_context(tc.tile_pool(name="singles", bufs=1))

        hid_sb = singles.tile([128, n_cc, S], BF16)
        wg_sb = singles.tile([128, 3, n_cc, 128], BF16)
        wv_sb = singles.tile([128, n_cc, HPC * HD], BF16)
        emb_sb = singles.tile([128, NE], BF16)
        smT_sb = singles.tile([128, 8, S], FP8)
        selw_sb = singles.tile([128, 8, 2], F32)
        qkT_sb = singles.tile([128, 3, S], BF16)
        k2lo_sb = singles.tile([64, S], BF16)
        v4t_sb = singles.tile([128, 8, HPC * HD], BF16)
        ones1 = singles.tile([128, 1], BF16)
        id8z = singles.tile([128, 2, 128], FP8)  # [identity | zeros]
        osb = singles.tile([128, LTN, HPC * HD], F32)

        hid_v = hidT.rearrange("(cc p) l -> p cc l", p=128)
        wg_v = wg.rearrange("g (cc p) d -> p g cc d", p=128)
        # priority order on the serial DMA resource: weights for G0, hid,
        # emb (phase R), wg1/wv; the first smT quarter and wg2/smT-rest are
        # queued around head 0's skew round trip so S(0) can start early.
        nc.sync.dma_start(out=wg_sb[:, 0], in_=wg_v[:, 0])
        cch = n_cc // 2
        nc.sync.dma_start(out=hid_sb[:, 0:cch], in_=hid_v[:, 0:cch])
        nc.sync.dma_start(out=hid_sb[:, cch:n_cc], in_=hid_v[:, cch:n_cc])
        nc.sync.dma_start(out=emb_sb, in_=embT2[:, :])
        nc.sync.dma_start(out=wg_sb[:, 1], in_=wg_v[:, 1])
        nc.sync.dma_start(out=wv_sb, in_=wv.rearrange("(cc p) d -> p cc d", p=128))
        if use_m:
            m_sb = singles.tile([128, 8, 2], F32)
            nc.sync.dma_start(out=m_sb, in_=mvec.rearrange("(rs p) w -> p rs w", p=128))

        nc.vector.memset(ones1, 1.0)
        nc.vector.memset(id8z[:, 1, :], 0.0)
        make_identity(nc, id8z[:, 0, :])
        junk = singles.tile([128, 16], BF16)
        nc.vector.memset(junk, 0.0)

        smT_v = smT.rearrange("(rs p) l -> p rs l", p=128)

        # R-phase psum pool lives across both P and S phases.
        ps_rb = ctx.enter_context(tc.tile_pool(name="ps_rb", bufs=3, space="PSUM"))
        rtp = ctx.enter_context(tc.tile_pool(name="rtp", bufs=3))
        bskp = ctx.enter_context(tc.tile_pool(name="bskp", bufs=3))
        sgtlp = ctx.enter_context(tc.tile_pool(name="sgtlp", bufs=3))
        pglp = ctx.enter_context(tc.tile_pool(name="pglp", bufs=3))
        finp = ctx.enter_context(tc.tile_pool(name="finp", bufs=2))

        rstate = {}
        ZS = LTN  # zeros slot index in bsk8 (for fp8 DoubleRow pairing)

        def r_begin(h):
            rt = rtp.tile([128, LTN, PITCH], FP8, tag="rt")
            bsk8 = bskp.tile([128, LTN + 1, S], FP8, tag="bsk")
            nc.gpsimd.memset(bsk8[:, ZS, :], 0.0)
            rstate[h] = (rt, bsk8)

        def r_unit(h, lt):
            """Band matmuls + psum->fp8 copies for one l-tile of head h.
            Head 0 runs in the P window where ACT is idle; later heads'
            copies go to DVE to keep ACT free for the exps."""
            rt, bsk8 = rstate[h]
            qg, qpo = QG[h]
            qb = qkT_sb[qpo:qpo + 64, qg, :]
            qbl = qb[:, lt * 128:(lt + 1) * 128]
            e0 = 896 - lt * 128
            pieces = ((0, 512), (512, 1024), (1024, BAND))
            for pi, (a, b) in enumerate(pieces):
                pr = ps_rb.tile([128, 512], F32, tag="rb")
                nc.tensor.matmul(
                    pr[:, 0:b - a], lhsT=qbl,
                    rhs=emb_sb[qpo:qpo + 64, e0 + a:e0 + b],
                    start=True, stop=True,
                )
                if h == 0:
                    if RT_H0 == "alt":
                        on_act = (lt + pi) % 2 == 0
                    else:
                        on_act = pi == 2 or (lt + pi) % 2 == 0
                elif RT_HS == "dve":
                    on_act = False
                elif RT_HS == "pi0":
                    on_act = pi == 0
                elif RT_HS == "pi2":
                    on_act = pi == 2
                elif RT_HS == "h1heavy":
                    on_act = pi in (0, 2) if h == 1 else pi == 0
                elif RT_HS == "h1all4":
                    if h == 1:
                        on_act = True if lt < 4 else pi in (0, 2)
                    else:
                        on_act = pi == 0
                else:
                    on_act = pi in (0, 2)
                if on_act:
                    nc.scalar.activation(rt[:, lt, a:b], pr[:, 0:b - a], COPY)
                else:
                    nc.vector.tensor_copy(rt[:, lt, a:b], pr[:, 0:b - a])
            if lt % 2 == 1:
                lt0 = lt - 1
                # the skew round trip gates the next S phase: dispatch these
                # ahead of bulk loads (smT/wg2) on the serial DMA resource
                with tc.high_priority():
                    # skew write: rows (lt, l_loc), cols [0, BAND), pitch PITCH
                    wview = _dram_ap(
                        skews[h][:], lt0 * 128 * PITCH,
                        [[PITCH, 128], [128 * PITCH, 2], [1, BAND]])
                    nc.sync.dma_start(out=wview, in_=rt[:, lt0:lt0 + 2, 0:BAND])
                    # skewed read-back: bias[l, r] per l-tile
                    rview = _dram_ap(
                        skews[h][:], lt0 * 128 * PITCH + 127,
                        [[BAND, 128], [128 * PITCH, 2], [1, S]])
                    nc.sync.dma_start(out=bsk8[:, lt0:lt0 + 2, :], in_=rview)

        def emit_G(g, pool, tag="pt", on_act=True):
            for n in range(2):
                pt = pool.tile([128, 512], F32, tag=tag)
                for cc in range(n_cc):
                    nc.tensor.matmul(
                        pt,
                        lhsT=wg_sb[:, g, cc, :],
                        rhs=hid_sb[:, cc, n * 512:(n + 1) * 512],
                        start=(cc == 0), stop=(cc == n_cc - 1),
                    )
                if on_act:
                    nc.scalar.activation(qkT_sb[:, g, n * 512:(n + 1) * 512],
                                         pt, COPY)
                else:
                    nc.vector.tensor_copy(qkT_sb[:, g, n * 512:(n + 1) * 512],
                                          pt)

        # ---- phase P: projections (G2 deferred into S(0): wg2 loads late) ----
        with tc.tile_pool(name="ps_pt", bufs=2, space="PSUM") as ps_pt:
            # burn the PE p-state ramp on junk matmuls while hid loads, so
            # the projections and band matmuls run at full clock
            for _ in range(N_WARM):
                pw = ps_pt.tile([128, 512], F32, tag="pt")
                nc.tensor.matmul(pw[0:16, :], lhsT=junk,
                                 rhs=_bcast(junk[:, :], 32, 1),
                                 start=True, stop=True)
            emit_G(0, ps_pt)
            nc.sync.dma_start(out=smT_sb[:, 0:2], in_=smT_v[:, 0:2])
            for lt in range(LTN):
                if lt == 0:
                    r_begin(0)
                r_unit(0, lt)
            emit_G(1, ps_pt, on_act=False)
            with tc.tile_wait_until(HOLD_MS):
                nc.sync.dma_start(out=smT_sb[:, 2:8], in_=smT_v[:, 2:8])
            for rs in range(8):
                pv = ps_pt.tile([128, HPC * HD], F32, tag="pt")
                for cc in range(n_cc):
                    nc.tensor.matmul(
                        pv,
                        lhsT=hid_sb[:, cc, rs * 128:(rs + 1) * 128],
                        rhs=wv_sb[:, cc, :],
                        start=(cc == 0), stop=(cc == n_cc - 1),
                    )
                nc.vector.tensor_copy(v4t_sb[:, rs, :], pv)
            with tc.tile_wait_until(HOLD_MS):
                nc.sync.dma_start(out=wg_sb[:, 2], in_=wg_v[:, 2])
                nc.sync.dma_start(out=selw_sb,
                                  in_=selw.rearrange("(lc p) w -> p lc w",
                                                     p=128))

        # ---- phase S + F, with R(h+1) interleaved into S(h) ----
        with tc.tile_pool(name="ps_s", bufs=2, space="PSUM") as ps_s, \
             tc.tile_pool(name="ps_pc", bufs=1, space="PSUM") as ps_pc:

            id8z_pair = id8z[:, :, :]

            def bias_acc(st_slice, bsk8, lt, rs, stop):
                """st_slice += bias_tile^T via a regular fp8 matmul with an
                identity rhs; the zeros slot pairs the lhsT/rhs so fp8
                DoubleRow mode runs 2 rows/cycle."""
                base = bsk8[:, lt, rs * 128:(rs + 1) * 128]
                lhsT = bass.AP(
                    tensor=base.tensor, offset=base.offset,
                    ap=[list(base.ap[0]), [(ZS - lt) * S, 2]]
                       + [list(d) for d in base.ap[1:]])
                nc.tensor.matmul(
                    st_slice, lhsT=lhsT, rhs=id8z_pair,
                    start=False, stop=stop,
                    perf_mode=mybir.MatmulPerfMode.DoubleRow,
                )

            def s_front(h, rs):
                """scores + bias + (s+b) staging + local mul + fused exp ->
                pgl for one r-chunk."""
                _, bsk8 = rstate[h]
                qg, qpo = QG[h]
                qf = qkT_sb[qpo:qpo + 64, qg, :]
                if h < 2:
                    kg, kpo = KG[h]
                    kf = qkT_sb[kpo:kpo + 64, kg, :]
                else:
                    kf = k2lo_sb[:, :]
                sgtl = sgtlp.tile([128, 2, S], BF16, tag="sgtl")
                for half in range(2):
                    st = ps_s.tile([128, 512], F32, tag="st")
                    nc.tensor.matmul(
                        st,
                        lhsT=kf[:, rs * 128:(rs + 1) * 128],
                        rhs=qf[:, half * 512:(half + 1) * 512],
                        start=True, stop=False,
                    )
                    for lc4 in range(4):
                        bias_acc(st[:, lc4 * 128:(lc4 + 1) * 128],
                                 bsk8, half * 4 + lc4, rs,
                                 stop=(lc4 == 3))
                    sg = sgtl[:, 0, half * 512:(half + 1) * 512]
                    # (s+b) psum -> SBUF bf16; mostly DVE (ACT is exp-bound)
                    stc_act = STC_MOD > 0 and \
                        (h * 16 + rs * 2 + half) % STC_MOD == 0
                    if use_m:
                        if stc_act:
                            nc.scalar.activation(
                                sg, st,
                                mybir.ActivationFunctionType.Identity,
                                bias=m_sb[:, rs, 1:2])
                        else:
                            nc.vector.tensor_scalar_add(
                                sg, st, m_sb[:, rs, 1:2])
                    elif stc_act:
                        nc.scalar.activation(sg, st, COPY)
                    else:
                        nc.vector.tensor_copy(sg, st)
                    nc.gpsimd.tensor_mul(
                        sgtl[:, 1, half * 512:(half + 1) * 512], sg,
                        smT_sb[:, rs, half * 512:(half + 1) * 512])
                pgl = pglp.tile([128, 2, S], BF16, tag="pgl")
                if use_m:
                    nc.scalar.activation(pgl[:, 0, :], sgtl[:, 0, :],
                                         EXP, scale=0.125)
                    nc.scalar.activation(pgl[:, 1, :], sgtl[:, 1, :],
                                         EXP, scale=0.125,
                                         bias=m_sb[:, rs, 0:1])
                else:
                    nc.scalar.activation(pgl, sgtl, EXP, scale=0.125)
                return pgl

            def emit_S(h, front0=None):
                vv = v4t_sb.rearrange("p rs (h d) -> p rs h d", d=HD)[:, :, h, :]
                pcv = ps_pc.tile([128, LTN, 2, HD], F32, tag="pcv")
                pcs = ps_pc.tile([128, LTN, 2, 1], F32, tag="pcs")
                next_front = None
                for rs in range(8):
                    if front0 is not None and rs < len(front0):
                        pgl = front0[rs]
                    else:
                        pgl = s_front(h, rs)
                    if rs == 7 and h + 1 < HPC:
                        # cross-head pipelining: next head's first front runs
                        # its exp while this head's last ctx waits
                        next_front = [s_front(h + 1, 0)]
                    # pcv spans 2 psum banks (lt 0-3 / 4-7); pcs one more.
                    # start_tensor_calc arms zero-on-write per 2KB bank, so
                    # exactly one start per bank and one stop at the bank's
                    # last matmul (PE executes in emission order).
                    for br in range(2):
                        for lt in range(LTN):
                            lw = pgl[:, br, lt * 128:(lt + 1) * 128]
                            nc.tensor.matmul(
                                pcv[:, lt, br, :], lhsT=lw,
                                rhs=vv[:, rs, :],
                                start=(rs == 0 and br == 0 and lt % 4 == 0),
                                stop=(rs == 7 and br == 1 and lt % 4 == 3),
                                skip_group_check=True,
                            )
                            nc.tensor.matmul(
                                pcs[:, lt, br, :], lhsT=lw,
                                rhs=ones1,
                                start=(rs == 0 and br == 0 and lt == 0),
                                stop=(rs == 7 and br == 1 and lt == 7),
                                skip_group_check=True,
                            )
                    if h == 0 and rs == 1:
                        emit_G(2, ps_s, tag="st", on_act=False)
                        # k2 re-based to partitions 0-63 (engines cannot
                        # cross partitions)
                        nc.sync.dma_start(out=k2lo_sb, in_=qkT_sb[64:128, 2, :])
                    if h + 1 < HPC and rs < 4:
                        if rs == 0:
                            r_begin(h + 1)
                        r_unit(h + 1, 2 * rs)
                        r_unit(h + 1, 2 * rs + 1)
                return pcv, pcs, next_front

            def emit_F(h, pcv, pcs):
                pcb = finp.tile([128, LTN, 2, HD], F32, tag="pcb")
                rsum = finp.tile([128, LTN, 2], F32, tag="rsum")
                w = finp.tile([128, LTN, 2], F32, tag="w")
                tmp = finp.tile([128, LTN, HD], F32, tag="tmp")
                tmp2 = finp.tile([128, LTN, HD], F32, tag="tmp2")
                ov = osb.rearrange("p lc (h d) -> p lc h d", d=HD)[:, :, h, :]
                for q in range(2):
                    a, b = q * 4, q * 4 + 4
                    nc.vector.tensor_copy(pcb[:, a:b], pcv[:, a:b])
                    nc.vector.reciprocal(rsum[:, a:b], pcs[:, a:b, :, 0])
                    nc.gpsimd.tensor_mul(w[:, a:b], rsum[:, a:b],
                                         selw_sb[:, a:b])
                    nc.gpsimd.tensor_mul(tmp[:, a:b], pcb[:, a:b, 1, :],
                                         _bcast(w[:, a:b, 1], HD, 2))
                    nc.gpsimd.tensor_mul(tmp2[:, a:b], pcb[:, a:b, 0, :],
                                         _bcast(w[:, a:b, 0], HD, 2))
                    nc.gpsimd.tensor_add(ov[:, a:b], tmp[:, a:b],
                                         tmp2[:, a:b])
                    outv = outp.rearrange("(lc p) c -> p lc c", p=128)
                    nc.sync.dma_start(
                        out=outv[:, a:b, h * HD:(h + 1) * HD],
                        in_=osb[:, a:b, h * HD:(h + 1) * HD])

            front = None
            for h in range(HPC):
                pcv, pcs, front = emit_S(h, front)
                emit_F(h, pcv, pcs)

    nc.compile()
    return nc


def _get_program(n_cc, use_m):
    key = (n_cc, use_m)
    if key not in _programs:
        _programs[key] = build_program(n_cc, use_m)
    return _programs[key]


def kernel(hidden_states, attention_mask, scaled_attention_mask, selector_outputs,
           Wq, bq, Wk, bk, Wv, bv, dist_emb):
    hidden_states = np.asarray(hidden_states, np.float32)
    attention_mask = np.asarray(attention_mask, np.float32)
    scaled_attention_mask = np.asarray(scaled_attention_mask, np.float32)
    selector_outputs = np.asarray(selector_outputs, np.float32)
    Wq, Wk, Wv = (np.asarray(x, np.float32) for x in (Wq, Wk, Wv))
    bq, bk, bv = (np.asarray(x, np.float32) for x in (bq, bk, bv))
    dist_emb = np.asarray(dist_emb, np.float32)

    use_bias = bool(np.any(bq) or np.any(bk) or np.any(bv))
    use_m = bool(np.any(attention_mask))
    n_cc = 7 if use_bias else 6
    CH = n_cc * 128
    nc = _get_program(n_cc, use_m)

    smT = np.ascontiguousarray(scaled_attention_mask[0, 0].T).astype(ml_dtypes.float8_e4m3)
    e_rev_t = dist_emb[::-1].T.astype(NPBF16)
    embT2 = np.ascontiguousarray(np.concatenate([e_rev_t, e_rev_t], axis=0))

    in_maps = []
    for core in range(NCORES):
        b = core // 4
        k4 = core % 4
        heads = [3 * k4, 3 * k4 + 1, 3 * k4 + 2]

        hidT = hidden_states[b].T  # [768, S]
        if use_bias:
            hidT = np.concatenate(
                [hidT, np.ones((1, S), np.float32),
                 np.zeros((CH - HID - 1, S), np.float32)], axis=0)
        hidT_bf = np.ascontiguousarray(hidT).astype(NPBF16)

        def wcols(W, bvec, h):
            c = W[:, h * HD:(h + 1) * HD]
            if use_bias:
                c = np.concatenate(
                    [c, bvec[None, h * HD:(h + 1) * HD],
                     np.zeros((CH - HID - 1, HD), np.float32)], axis=0)
            return c

        q0, q1, q2 = (wcols(Wq, bq, h) for h in heads)
        k0, k1, k2 = (wcols(Wk, bk, h) for h in heads)
        wg_np = np.stack([
            np.concatenate([q0, q1], axis=1),
            np.concatenate([k0, k1], axis=1),
            np.concatenate([q2, k2], axis=1),
        ]).astype(NPBF16)
        wv_np = np.concatenate(
            [wcols(Wv, bv, h) for h in heads], axis=1).astype(NPBF16)

        sel = selector_outputs[b, 0, :, 0]
        selw_np = np.stack([1.0 - sel, sel], axis=1).astype(np.float32)

        m = {
            "hidT": hidT_bf,
            "wg": wg_np,
            "wv": np.ascontiguousarray(wv_np),
            "embT2": embT2,
            "smT": smT,
            "selw": np.ascontiguousarray(selw_np),
        }
        if use_m:
            mv = attention_mask[b, 0, 0]
            m["mvec"] = np.ascontiguousarray(
                np.stack([mv, 8.0 * mv], axis=1).astype(np.float32))
        in_maps.append(m)

    res = run_bass_kernel_spmd(nc, in_maps, list(range(NCORES)))

    out = np.empty((B, S, HID), np.float32)
    for core in range(NCORES):
        b = core // 4
        k4 = core % 4
        out[b, :, 192 * k4:192 * (k4 + 1)] = res.results[core]["out"]
    return out
